# revision 39
# baseline (speedup 1.0000x reference)
"""Differential attention (B=2, S=2048, HS=1024, H=16, KV=4, D=64) on 8 trn2 cores.

Sharding: core c = (b, g) with b = c // 4 (data parallel on batch) and
g = c % 4 (tensor parallel over the 4 KV head groups; each core owns the
4 query heads of its group).  Each core computes its 4 heads' normed
attention output and a row-parallel partial of the output projection
(out_pt = (O_heads @ Wo_rows)^T, bf16); the host upcasts and sums the 4
partials per batch.

QKV projections run in fp8e4(DoubleRow): x and Wq/Wk/Wv are quantized to
e4m3 and hs-chunk PAIRS are contracted per instruction (0.5 cycles/row
at doubled contraction = 4x over bf16).  S and U matmuls stay bf16: that
keeps the PE queue about as busy as the ACT engine (the exp stream is
the intrinsic floor), which matters because an idle PE loses its p-state
ramp.

RoPE without the permutation matmul: rot(q)[d] = ∓q[(d±32) mod 64-block]
is four 32-partition-shifted DVE multiplies against a sign-folded sin
table (32-multiple partition shifts are legal on DVE); q*cos and the
final add run on the otherwise idle Pool engine.

attention(qt), per head: flash-style causal attention over k tiles,
S^T[k,q] strips via two 64-contraction matmuls, P = exp(S/8) on ACT
(no row-max: scores are O(5); diagonal strips exp both branch segments
with ONE strided-AP activation), STAGE=8 k-tiles of S/exp ahead of the
U matmuls, causal wedge zeroed by one dual-block mask-multiply on Pool,
U^T[128,q] += [V|ones].T @ P — the ones block replicates the softmax
denominators onto partitions 64..127.  Epilogue needs one reciprocal
instead of two: out is rms-normalized, so O~ = U1 - lam~*U2 with
lam~ = lam*d1/d2 (a per-query row vector built on the 64 redundant
denominator partitions with reciprocal + scalar_tensor_tensor) has the
same normalized value as O = U1/d1 - lam*U2/d2.

rms: each head's ssq row-sum lands on partition 32j of a [128, QTW]
tile, so one Ln and one Exp over partitions 0..96 (ACT cost is free-size
only) replace per-head activations; Ln/Exp share the preloaded softmax
act table.  Pool also does the rms broadcasts and the onq multiplies.

Pipeline: proj(qt+1) units fill attention(qt); rms(qt-1) at head slot 1,
wo(qt-1) halves at head slots 2/3.  PSUM: psS pairs [128,1024]
double-buffered (4 banks) + psU [128,1024] single (2) + aux ring (2).
"""

import math
import os
import sys

import numpy as np

try:
    import concourse.bass as bass  # noqa: F401
except ImportError:
    sys.path.insert(0, "/opt/trn_rl_repo")

import concourse.bass as bass
import concourse.tile as tile
from concourse import bacc, mybir
from concourse import bass_utils

f32 = mybir.dt.float32
bf16 = mybir.dt.bfloat16
fp8 = mybir.dt.float8e4
AF = mybir.ActivationFunctionType
ALU = mybir.AluOpType
DR = mybir.MatmulPerfMode.DoubleRow

B, S, HS = 2, 2048, 1024
H, KV, D = 16, 4, 64
NHL = 4            # query heads per core
NQT = 4            # q tiles of 512
QTW = 512
NKT = 16           # k tiles of 128
NHS = 8            # hs tiles of 128
NEG = -1e9
EPS = 1e-5

_prog_cache = {}
PHASE_LOG = []
_F = os.environ.get
FP8_QKV = _F("K_FP8_QKV", "0") == "1"
SIMPLE_IN = _F("K_SIMPLE_IN", "0") == "1"
POOL_OFF = _F("K_POOL_OFF", "0") == "1"   # 1: everything back on DVE
DEBUG_DUMP = _F("K_DEBUG_DUMP", "0") == "1"
LAMT_EPI = _F("K_LAMT_EPI", "0") == "1"   # single-recip lam~ epilogue
                                          # (breaks the r1/r2 error
                                          # correlation; adds ~3% err)
OLD_RMS = _F("K_OLD_RMS", "0") == "1"     # per-head [1,W] Ln/Exp
OLD_DIAG = _F("K_OLD_DIAG", "0") == "1"   # two-instr diag exp
OLD_ROPE = _F("K_OLD_ROPE", "0") == "1"   # perm-matmul rot
DUMP_LITE = _F("K_DUMP_LITE", "0") == "1"


def _build_program(lam: float):
    nc = bacc.Bacc("TRN2", target_bir_lowering=False, debug=False,
                   enable_asserts=False, num_devices=8)
    PHASE_LOG.clear()

    def mark(label):
        PHASE_LOG.append((label, nc.next_id()))

    in_dt = fp8 if FP8_QKV else bf16
    xt = nc.dram_tensor("xt", [HS, S], in_dt, kind="ExternalInput").ap()
    wq = nc.dram_tensor("wq", [HS, 512], in_dt, kind="ExternalInput").ap()
    wk = nc.dram_tensor("wk", [HS, 128], in_dt, kind="ExternalInput").ap()
    wv = nc.dram_tensor("wv", [HS, 64], in_dt, kind="ExternalInput").ap()
    wo = nc.dram_tensor("wo", [256, HS], bf16, kind="ExternalInput").ap()
    perm = nc.dram_tensor("perm", [128, 128], bf16, kind="ExternalInput").ap()
    cos_t = nc.dram_tensor("cos_t", [128, S], bf16, kind="ExternalInput").ap()
    sin_t = nc.dram_tensor("sin_t", [128, S], bf16, kind="ExternalInput").ap()
    trimask = nc.dram_tensor("trimask", [128, 128], bf16,
                             kind="ExternalInput").ap()
    out_pt = nc.dram_tensor("out_pt", [HS, S], bf16, kind="ExternalOutput").ap()
    dbg = {}
    if DUMP_LITE:
        for nm, shp in (("dbg_ssqr0", [128, 512]), ("dbg_rmq0", [128, 512]),
                        ("dbg_on0", [128, 512]), ("dbg_on1", [128, 512])):
            dbg[nm] = nc.dram_tensor(nm, shp, f32, kind="ExternalOutput").ap()
    if DEBUG_DUMP:
        for nm, shp in (("dbg_k", [128, S]), ("dbg_q00", [128, 512]),
                        ("dbg_op0", [128, 512]), ("dbg_on0", [128, 512]),
                        ("dbg_ssqr0", [128, 512]), ("dbg_psu00", [128, 1024]),
                        ("dbg_va0", [128, 128])):
            dbg[nm] = nc.dram_tensor(nm, shp, f32, kind="ExternalOutput").ap()

    pool_eng = nc.vector if POOL_OFF else nc.gpsimd

    with tile.TileContext(nc) as tc:
        with tc.tile_pool(name="persist", bufs=1) as pp, \
             tc.tile_pool(name="loc", bufs=3) as loc, \
             tc.tile_pool(name="pwk", bufs=3) as pwk, \
             tc.tile_pool(name="patt", bufs=16) as pa, \
             tc.tile_pool(name="ep", bufs=4) as pe, \
             tc.tile_pool(name="rmsp", bufs=2) as prm, \
             tc.psum_pool(name="ps", bufs=2) as ps_:

            # preload the act-func set that holds BOTH Exp and Ln so the
            # table-load pass never has to switch sets mid-stream
            from concourse.hw_specs import get_activation_tables
            _tables = list(get_activation_tables(nc.m.arch).items())
            _set_id = next(i for i, (_, fs) in enumerate(_tables)
                           if AF.Exp in fs and AF.Ln in fs)
            _ld = mybir.InstLoadActFuncSet(
                name=nc.get_next_instruction_name(),
                act_func_set_id=_set_id, ins=[], outs=[])
            nc.scalar.add_instruction(_ld)

            W = {}
            warm = pp.tile([64, 64], bf16, name="warm", tag="warm")
            nc.vector.memset(warm[:], 1.0)

            def emit_warm(n, dep=None):
                # tiny keep-alive matmuls: hold the PE p-state ramp through
                # windows where no real matmul is ready; `dep` staggers the
                # batch behind a chain-produced tile
                psd = ps_.tile([128, 2 * QTW], f32, name="psd", tag="psS")
                lhs = warm[0:64, 0:1] if dep is None else dep[:, 0:1]
                rhs = warm[0:64, 0:64] if dep is None else dep[:, 0:64]
                for _ in range(n):
                    nc.tensor.matmul(psd[0:1, 0:64], lhs, rhs,
                                     start=True, stop=True,
                                     skip_group_check=True)

            def emit_weight_loads0(state):
                W['wk_a'] = pp.tile([128, 8 * 128], in_dt, name="wk", tag="wk")
                if SIMPLE_IN:
                    for hs in range(NHS):
                        nc.sync.dma_start(
                            W['wk_a'][:, hs * 128:(hs + 1) * 128],
                            wk[hs * 128:(hs + 1) * 128, :])
                else:
                    nc.sync.dma_start(
                        W['wk_a'][:].rearrange("p (h c) -> p h c", h=NHS),
                        wk[:].rearrange("(h p) c -> p h c", h=NHS))
                emit_xt_loads(0, state)
                W['perm_sb'] = pp.tile([128, 128], bf16, name="perm",
                                       tag="perm")
                nc.scalar.dma_start(W['perm_sb'][:], perm[:])
                W['cos_sb'] = pp.tile([128, S], bf16, name="cos", tag="cos")
                nc.scalar.dma_start(W['cos_sb'][:], cos_t[:])
                W['sin_sb'] = pp.tile([128, S], bf16, name="sin", tag="sin")
                nc.sync.dma_start(W['sin_sb'][:], sin_t[:])
                W['wq_a'] = pp.tile([128, 8 * 512], in_dt, name="wq", tag="wq")
                if SIMPLE_IN:
                    for hs in range(NHS):
                        nc.sync.dma_start(
                            W['wq_a'][:, hs * 512:(hs + 1) * 512],
                            wq[hs * 128:(hs + 1) * 128, :])
                else:
                    for t in range(2):
                        nc.sync.dma_start(
                            W['wq_a'][:, t * 2048:(t + 1) * 2048].rearrange(
                                "p (h c) -> p h c", h=4),
                            wq[t * 512:(t + 1) * 512, :].rearrange(
                                "(h p) c -> p h c", h=4))
                W['wv_a'] = pp.tile([128, 8 * 64], in_dt, name="wv", tag="wv")
                if SIMPLE_IN:
                    for hs in range(NHS):
                        nc.scalar.dma_start(
                            W['wv_a'][:, hs * 64:(hs + 1) * 64],
                            wv[hs * 128:(hs + 1) * 128, :])
                else:
                    nc.scalar.dma_start(
                        W['wv_a'][:].rearrange("p (h c) -> p h c", h=NHS),
                        wv[:].rearrange("(h p) c -> p h c", h=NHS))
                W['ones_sb'] = pp.tile([128, 64], bf16, name="ones", tag="ones")
                nc.vector.memset(W['ones_sb'][:], 1.0)
                W['tri_sb'] = pp.tile([128, 128], bf16, name="tri", tag="tri")
                nc.sync.dma_start(W['tri_sb'][:], trimask[:])
                W['wo_sb'] = []
                for t in range(2):
                    w = pp.tile([128, HS], bf16, name=f"wo{t}", tag=f"wo{t}")
                    nc.scalar.dma_start(w[:], wo[t * 128:(t + 1) * 128, :])
                    W['wo_sb'].append(w)
            k_sb = pp.tile([128, S], bf16, name="k", tag="k")
            va = [pp.tile([128, 128], bf16, name=f"va{kt}", tag=f"va{kt}")
                  for kt in range(NKT)]
            for kt in range(NKT):
                nc.vector.memset(va[kt][:, 64:128], 1.0)
            eps_sb = pp.tile([128, 1], f32, name="eps", tag="eps")
            nc.vector.memset(eps_sb[:], EPS)

            def rope_block(ps, dst, qlo, qhi, dst_sb=None):
                # dst = q*cos + rot(q)*sin; rot = 4 partition-32-shifted
                # multiplies.  Both SBUF inputs of a TensorTensor must share
                # their base partition, so the sin table is row-permuted on
                # the host (sin_sb[p] holds the signed sin of the PARTNER row
                # p+-32); qc/add run on Pool
                q_sb = dst_sb
                if q_sb is None:
                    q_sb = pwk.tile([128, QTW], bf16, name="qsb", tag="qsb")
                nc.vector.tensor_copy(q_sb[:], ps[:])
                qc = pwk.tile([128, QTW], bf16, name="qc", tag="qc")
                pool_eng.tensor_mul(qc[:], q_sb[:], W['cos_sb'][:, qlo:qhi])
                if OLD_ROPE:
                    psr = ps_.tile([128, QTW], f32, name="psr", tag="aux")
                    nc.tensor.matmul(psr[:], W['perm_sb'][:], q_sb[:],
                                     start=True, stop=True)
                    qs0 = pwk.tile([128, QTW], bf16, name="qs", tag="qs")
                    nc.vector.tensor_mul(qs0[:], psr[:],
                                         W['sin_sb'][:, qlo:qhi])
                    pool_eng.tensor_add(dst, qc[:], qs0[:])
                    return
                qs = pwk.tile([128, QTW], bf16, name="qs", tag="qs")
                for blk in range(2):
                    b0 = 64 * blk
                    nc.vector.tensor_mul(qs[b0:b0 + 32, :],
                                         q_sb[b0 + 32:b0 + 64, :],
                                         W['sin_sb'][b0 + 32:b0 + 64,
                                                     qlo:qhi])
                    nc.vector.tensor_mul(qs[b0 + 32:b0 + 64, :],
                                         q_sb[b0:b0 + 32, :],
                                         W['sin_sb'][b0:b0 + 32, qlo:qhi])
                pool_eng.tensor_add(dst, qc[:], qs[:])

            def emit_xt_loads(qt, state):
                mark(f"xt{qt}")
                qlo, qhi = qt * QTW, (qt + 1) * QTW
                xt_a = pwk.tile([128, 8 * QTW], in_dt, name="xt", tag="xt")
                if SIMPLE_IN:
                    for hs in range(NHS):
                        eng = nc.scalar if hs % 2 == 0 else nc.sync
                        eng.dma_start(xt_a[:, hs * QTW:(hs + 1) * QTW],
                                      xt[hs * 128:(hs + 1) * 128, qlo:qhi])
                else:
                    for t in range(2):
                        eng = nc.scalar if t == 0 else nc.sync
                        eng.dma_start(
                            xt_a[:, t * 2048:(t + 1) * 2048].rearrange(
                                "p (h c) -> p h c", h=4),
                            xt[t * 512:(t + 1) * 512, qlo:qhi].rearrange(
                                "(h p) c -> p h c", h=4))
                state[(qt, "xt")] = xt_a

            def proj_units(qt, state):
                mark(f"proj{qt}")
                qlo, qhi = qt * QTW, (qt + 1) * QTW
                xt_a = state[(qt, "xt")]
                psk = ps_.tile([128, QTW], f32, name="psk", tag="aux")
                if FP8_QKV:
                    xt_c = xt_a[:].rearrange("p (c n) -> p c n", c=NHS)
                    wk_c = W['wk_a'][:].rearrange("p (c n) -> p c n", c=NHS)
                    wq_c = W['wq_a'][:].rearrange("p (c n) -> p c n", c=NHS)
                    wv_c = W['wv_a'][:].rearrange("p (c n) -> p c n", c=NHS)
                    for u in range(4):
                        nc.tensor.matmul(
                            psk[:], wk_c[:, 2 * u:2 * u + 2, :],
                            xt_c[:, 2 * u:2 * u + 2, :],
                            start=(u == 0), stop=(u == 3), perf_mode=DR)
                        if u % 2 == 1:
                            yield
                else:
                    for half_ in range(2):
                        for hs in range(4 * half_, 4 * half_ + 4):
                            nc.tensor.matmul(
                                psk[:], W['wk_a'][:, hs * 128:(hs + 1) * 128],
                                xt_a[:, hs * 512:(hs + 1) * 512],
                                start=(hs == 0), stop=(hs == NHS - 1))
                        yield
                rope_block(psk, k_sb[:, qlo:qhi], qlo, qhi)
                yield
                qloc = [loc.tile([128, QTW], bf16, name=f"q{jj}", tag=f"q{jj}")
                        for jj in range(NHL)]
                state[qt] = qloc
                for j in range(NHL):
                    psq = ps_.tile([128, QTW], f32, name="psq", tag="aux")
                    if FP8_QKV:
                        for u in range(4):
                            nc.tensor.matmul(
                                psq[:],
                                wq_c[:, 2 * u:2 * u + 2,
                                     j * 128:(j + 1) * 128],
                                xt_c[:, 2 * u:2 * u + 2, :],
                                start=(u == 0), stop=(u == 3), perf_mode=DR)
                            if u % 2 == 1:
                                yield
                    else:
                        for half_ in range(2):
                            for hs in range(4 * half_, 4 * half_ + 4):
                                nc.tensor.matmul(
                                    psq[:],
                                    W['wq_a'][:, hs * 512 + j * 128:
                                              hs * 512 + (j + 1) * 128],
                                    xt_a[:, hs * 512:(hs + 1) * 512],
                                    start=(hs == 0), stop=(hs == NHS - 1))
                            yield
                    rope_block(psq, qloc[j][:], qlo, qhi)
                    yield
                for kk in range(4):
                    kt = 4 * qt + kk
                    psvt = ps_.tile([128, 64], f32, name="psvt", tag="aux")
                    if FP8_QKV:
                        for u in range(4):
                            nc.tensor.matmul(
                                psvt[:],
                                xt_c[:, 2 * u:2 * u + 2,
                                     kk * 128:(kk + 1) * 128],
                                wv_c[:, 2 * u:2 * u + 2, :],
                                start=(u == 0), stop=(u == 3), perf_mode=DR)
                    else:
                        for hs in range(NHS):
                            nc.tensor.matmul(
                                psvt[:],
                                xt_a[:, hs * 512 + kk * 128:
                                     hs * 512 + (kk + 1) * 128],
                                W['wv_a'][:, hs * 64:(hs + 1) * 64],
                                start=(hs == 0), stop=(hs == NHS - 1))
                    nc.vector.tensor_copy(va[kt][:, 0:64], psvt[:])
                    yield

            def emit_att_head(qt, j, state, filler=None):
                mark(f"att{qt}.{j}")
                qloc = state[qt]
                if j == 0:
                    state[(qt, "op")] = [loc.tile([128, QTW], bf16,
                                                  name=f"op{t}", tag=f"op{t}")
                                         for t in range(2)]
                    state[(qt, "on")] = [loc.tile([128, QTW], bf16,
                                                  name=f"on{t}", tag=f"on{t}")
                                         for t in range(2)]
                opair = state[(qt, "op")]
                half, pt = (j % 2) * 64, j // 2
                last_kt = 4 * qt + 3
                psu = ps_.tile([128, 2 * QTW], f32, name="psu", tag="psU",
                               bufs=1)
                p12s = {}

                def emit_s_exp(kt):
                    jd = kt - 4 * qt
                    q0 = 128 * jd if jd >= 0 else 0
                    pss = ps_.tile([128, 2 * QTW], f32, name="pss", tag="psS")
                    nc.tensor.matmul(
                        pss[:, q0:QTW],
                        k_sb[0:64, kt * 128:(kt + 1) * 128],
                        qloc[j][0:64, q0:QTW],
                        start=True, stop=True, skip_group_check=True)
                    nc.tensor.matmul(
                        pss[:, QTW + q0:2 * QTW],
                        k_sb[64:128, kt * 128:(kt + 1) * 128],
                        qloc[j][64:128, q0:QTW],
                        start=True, stop=True, skip_group_check=True)
                    p12 = pa.tile([128, 2 * QTW], bf16, name="p12", tag="p12")
                    if q0 == 0:
                        nc.scalar.activation(p12[:, 0:2 * QTW],
                                             pss[:, 0:2 * QTW],
                                             AF.Exp, scale=0.125)
                    elif OLD_DIAG:
                        nc.scalar.activation(p12[:, q0:QTW], pss[:, q0:QTW],
                                             AF.Exp, scale=0.125)
                        nc.scalar.activation(p12[:, QTW + q0:2 * QTW],
                                             pss[:, QTW + q0:2 * QTW],
                                             AF.Exp, scale=0.125)
                    else:
                        nc.scalar.activation(
                            p12[:].rearrange("p (b q) -> p b q",
                                             b=2)[:, :, q0:QTW],
                            pss[:].rearrange("p (b q) -> p b q",
                                             b=2)[:, :, q0:QTW],
                            AF.Exp, scale=0.125)
                    if jd >= 0:
                        wap = p12[:].rearrange("p (b q) -> p b q",
                                               b=2)[:, :, q0:q0 + 128]
                        msk = W['tri_sb'][:].unsqueeze(1).broadcast_to(
                            [128, 2, 128])
                        pool_eng.tensor_mul(wap, wap, msk)
                    p12s[kt] = p12

                def drain_ssq():
                    if state.get("pend_ssq") is None:
                        return
                    osq_, ssqr_, half_, j_ = state.pop("pend_ssq")
                    psss = ps_.tile([1, QTW], f32, name="psss", tag="aux")
                    nc.tensor.matmul(psss[:],
                                     W['ones_sb'][half_:half_ + 64, 0:1],
                                     osq_[half_:half_ + 64, :],
                                     start=True, stop=True,
                                     skip_group_check=True)
                    nc.vector.tensor_copy(
                        ssqr_[32 * j_:32 * j_ + 1, 0:QTW], psss[0:1, :])

                STAGE = 8
                for kt in range(min(STAGE, last_kt + 1)):
                    emit_s_exp(kt)
                drain_ssq()
                for kt in range(last_kt + 1):
                    if kt + STAGE <= last_kt:
                        emit_s_exp(kt + STAGE)
                    jd = kt - 4 * qt
                    q0 = 128 * jd if jd >= 0 else 0
                    p12 = p12s.pop(kt)
                    # lam lives in the epilogue now, so both branches share
                    # [V|ones] (a matmul output cannot span PSUM banks, so
                    # the two branch segments stay separate instructions)
                    nc.tensor.matmul(
                        psu[:, q0:QTW], va[kt][:], p12[:, q0:QTW],
                        start=(kt == 0), stop=(kt == last_kt),
                        skip_group_check=True)
                    nc.tensor.matmul(
                        psu[:, QTW + q0:2 * QTW], va[kt][:],
                        p12[:, QTW + q0:2 * QTW],
                        start=(kt == 0), stop=(kt == last_kt),
                        skip_group_check=True)

                if filler is not None:
                    for _ in range(5):
                        next(filler, None)
                if DEBUG_DUMP and qt == 0 and j == 0:
                    d_ = pe.tile([128, 1024], f32, name="dpsu", tag="dbgd")
                    nc.vector.tensor_copy(d_[:], psu[:])
                    nc.sync.dma_start(dbg["dbg_psu00"][:], d_[:])
                    for nm, t_ in (("dbg_k", k_sb), ("dbg_q00", qloc[0]),
                                   ("dbg_va0", va[0])):
                        p_, f_ = t_[:].shape
                        d2 = pe.tile([128, S], f32, name=f"d{nm}", tag="dbgd2",
                                     bufs=3)
                        nc.vector.tensor_copy(d2[0:p_, 0:f_], t_[:])
                        nc.sync.dma_start(dbg[nm][:, 0:f_], d2[0:p_, 0:f_])
                mark(f"epi{qt}.{j}")
                if LAMT_EPI:
                    # single-reciprocal O~ = U1 - (lam*d1/d2)*U2; the d1
                    # scale cancels in the rms.  Numerically worse: the two
                    # branches' reciprocal errors no longer cancel in the
                    # differential subtraction.
                    rec2 = pe.tile([64, QTW], f32, name="rec2", tag="rec2")
                    nc.vector.reciprocal(rec2[:], psu[64:128, QTW:2 * QTW])
                    lamt = pe.tile([64, QTW], f32, name="lamt", tag="lamt")
                    nc.vector.scalar_tensor_tensor(
                        lamt[:], rec2[:], lam, psu[64:128, 0:QTW],
                        ALU.mult, ALU.mult)
                    t2 = pe.tile([64, QTW], bf16, name="t2", tag="t2")
                    nc.vector.tensor_mul(t2[:], psu[0:64, QTW:2 * QTW],
                                         lamt[:])
                    nc.vector.tensor_sub(opair[pt][half:half + 64, :],
                                         psu[0:64, 0:QTW], t2[:])
                else:
                    # O = U1/d1 - lam*U2/d2: both reciprocals come from the
                    # same DVE approx, so their errors track (d1 ~= d2) and
                    # cancel in the differential subtraction
                    wri = pe.tile([64, 2 * QTW], f32, name="wri", tag="wri")
                    nc.vector.reciprocal(wri[:], psu[64:128, :])
                    t1 = pe.tile([64, QTW], bf16, name="t1", tag="t1")
                    nc.vector.tensor_mul(t1[:], psu[0:64, 0:QTW],
                                         wri[0:64, 0:QTW])
                    t2 = pe.tile([64, QTW], bf16, name="t2", tag="t2")
                    nc.vector.scalar_tensor_tensor(
                        t2[:], psu[0:64, QTW:2 * QTW], lam,
                        wri[0:64, QTW:2 * QTW], ALU.mult, ALU.mult)
                    nc.vector.tensor_sub(opair[pt][half:half + 64, :],
                                         t1[:], t2[:])
                # fused ssq contribution for the rms: row-sum of O~^2
                if j == 0:
                    ssqr = prm.tile([128, QTW], f32, name="ssqr", tag="ssqr")
                    pool_eng.memset(ssqr[:], 1.0)
                    state[(qt, "ssqr")] = ssqr
                ssqr = state[(qt, "ssqr")]
                osq = prm.tile([128, QTW], bf16, name="osq", tag="osq")
                nc.vector.tensor_mul(osq[half:half + 64, :],
                                     opair[pt][half:half + 64, :],
                                     opair[pt][half:half + 64, :])
                state["pend_ssq"] = (osq, ssqr, half, j)

            def emit_rms(qt, state, pts=(0, 1)):
                mark(f"rms{qt}")
                if state.get("pend_ssq") is not None:
                    osq_, ssqr_, half_, j_ = state.pop("pend_ssq")
                    psss = ps_.tile([1, QTW], f32, name="psss", tag="aux")
                    nc.tensor.matmul(psss[:],
                                     W['ones_sb'][half_:half_ + 64, 0:1],
                                     osq_[half_:half_ + 64, :],
                                     start=True, stop=True,
                                     skip_group_check=True)
                    nc.vector.tensor_copy(
                        ssqr_[32 * j_:32 * j_ + 1, 0:QTW], psss[0:1, :])
                opair = state[(qt, "op")]
                onq = state[(qt, "on")]
                ssqr = state[(qt, "ssqr")]
                # head j's ssq sits on partition 32j; one Ln + one Exp span
                # the whole partition range (ACT cost is free-size only; the
                # in-between rows hold memset filler)
                r0 = 0   # device activations start at partition 0
                r1 = 64 * pts[-1] + 33
                sqr = prm.tile([128, QTW], f32, name="sqr", tag="sqr")
                rmq = prm.tile([128, QTW], bf16, name="rmq", tag="rmq")
                if OLD_RMS:
                    for pt in pts:
                        for rr in (64 * pt, 64 * pt + 32):
                            nc.scalar.activation(sqr[rr:rr + 1, :],
                                                 ssqr[rr:rr + 1, :],
                                                 AF.Ln, scale=1.0 / 64.0,
                                                 bias=eps_sb[rr:rr + 1, 0:1])
                            nc.scalar.activation(rmq[rr:rr + 1, :],
                                                 sqr[rr:rr + 1, :],
                                                 AF.Exp, scale=-0.5)
                else:
                    nc.scalar.activation(sqr[r0:r1, :], ssqr[r0:r1, :],
                                         AF.Ln, scale=1.0 / 64.0,
                                         bias=eps_sb[r0:r1, 0:1])
                    nc.scalar.activation(rmq[r0:r1, :], sqr[r0:r1, :],
                                         AF.Exp, scale=-0.5)
                if DEBUG_DUMP and qt == 0:
                    dss = prm.tile([128, QTW], f32, name="dss", tag="dss")
                    nc.vector.tensor_copy(dss[:], ssqr[:])
                    nc.sync.dma_start(dbg["dbg_ssqr0"][:], dss[:])
                for pt in pts:
                    # partition_broadcast only honors a base-0 input on HW:
                    # stage the two factor rows onto partition 0 first
                    stg = prm.tile([1, 2 * QTW], bf16, name="stg", tag="stg",
                                   bufs=2)
                    nc.vector.tensor_copy(stg[0:1, 0:QTW],
                                          rmq[64 * pt:64 * pt + 1, :])
                    nc.vector.tensor_copy(stg[0:1, QTW:2 * QTW],
                                          rmq[64 * pt + 32:64 * pt + 33, :])
                    rsa = prm.tile([128, QTW], bf16, name="rsa", tag="rsa",
                                   bufs=2)
                    nc.gpsimd.partition_broadcast(rsa[:], stg[0:1, 0:QTW])
                    rsb = prm.tile([128, QTW], bf16, name="rsb", tag="rsb",
                                   bufs=2)
                    nc.gpsimd.partition_broadcast(rsb[:],
                                                  stg[0:1, QTW:2 * QTW])
                    pool_eng.tensor_mul(onq[pt][0:64, :], opair[pt][0:64, :],
                                        rsa[0:64, :])
                    pool_eng.tensor_mul(onq[pt][64:128, :],
                                        opair[pt][64:128, :], rsb[64:128, :])
                if DEBUG_DUMP and qt == 0:
                    for nm, t_ in (("dbg_op0", opair[0]), ("dbg_on0", onq[0])):
                        d_ = prm.tile([128, 512], f32, name=f"e{nm}",
                                      tag="dbge", bufs=2)
                        nc.vector.tensor_copy(d_[:], t_[:])
                        nc.sync.dma_start(dbg[nm][:], d_[:])
                if DUMP_LITE and qt == 0:
                    for nm, t_ in (("dbg_ssqr0", ssqr), ("dbg_rmq0", rmq),
                                   ("dbg_on0", onq[0]), ("dbg_on1", onq[1])):
                        d_ = prm.tile([128, 512], f32, name=f"L{nm}",
                                      tag="dbgL", bufs=4)
                        nc.vector.tensor_copy(d_[:], t_[:])
                        nc.sync.dma_start(dbg[nm][:], d_[:])

            def emit_wo(qt, state, half):
                mark(f"wo{qt}.{half}")
                qlo, qhi = qt * QTW, (qt + 1) * QTW
                onq = state[(qt, "on")]
                oc0 = 4 * half
                ow = prm.tile([128, 4 * QTW], bf16, name="ow", tag="ow")
                for i, oc in enumerate(range(oc0, oc0 + 4)):
                    psw = ps_.tile([128, QTW], f32, name="psw", tag="aux")
                    nc.tensor.matmul(
                        psw[:], W['wo_sb'][0][:, oc * 128:(oc + 1) * 128],
                        onq[0][:], start=True, stop=False)
                    nc.tensor.matmul(
                        psw[:], W['wo_sb'][1][:, oc * 128:(oc + 1) * 128],
                        onq[1][:], start=False, stop=True)
                    nc.vector.tensor_copy(ow[:, i * QTW:(i + 1) * QTW],
                                          psw[:])
                nc.sync.dma_start(
                    out_pt[oc0 * 128:(oc0 + 4) * 128,
                           qlo:qhi].rearrange("(b p) c -> p b c", b=4),
                    ow[:].rearrange("p (b c) -> p b c", b=4))

            state = {}
            emit_weight_loads0(state)
            for _ in proj_units(0, state):
                pass
            for qt in range(NQT):
                gen = None
                if qt < NQT - 1:
                    emit_xt_loads(qt + 1, state)
                    gen = proj_units(qt + 1, state)
                for j in range(NHL):
                    emit_att_head(qt, j, state, filler=gen)
                    if j == 1 and qt > 0:
                        emit_rms(qt - 1, state)
                    if j == 1 and qt == NQT - 1:
                        emit_rms(qt, state, pts=(0,))
                    if j == 2 and qt > 0:
                        emit_wo(qt - 1, state, half=0)
                    if j == 3 and qt > 0:
                        emit_wo(qt - 1, state, half=1)
                if gen is not None:
                    for _ in gen:
                        pass
            emit_rms(NQT - 1, state, pts=(1,))
            emit_wo(NQT - 1, state, half=0)
            emit_wo(NQT - 1, state, half=1)
    nc.compile()
    return nc


def get_program(lam: float):
    key = (round(float(lam), 9), FP8_QKV, SIMPLE_IN, POOL_OFF, DEBUG_DUMP,
           LAMT_EPI, OLD_RMS, OLD_DIAG, OLD_ROPE, DUMP_LITE)
    if key not in _prog_cache:
        _prog_cache[key] = _build_program(float(lam))
    return _prog_cache[key]


def _bf16():
    import ml_dtypes
    return ml_dtypes.bfloat16


def _fp8():
    import ml_dtypes
    return ml_dtypes.float8_e4m3


def _perm_mat():
    # For OLD_ROPE: psr = perm.T @ q with psr[d] = rot(q)[d]/sgn2(d) where
    # sgn2 is the sign folded into sin_t; both cases come out -1.
    p = np.zeros((128, 128), np.float32)
    for o in range(128):
        if o % 64 < 32:
            p[o + 32, o] = -1.0
        else:
            p[o - 32, o] = -1.0
    return p


def _host_inputs(x, rope_cos, rope_sin, Wq, Wk, Wv, Wo, subln_w, lam):
    bf = _bf16()
    in_t = _fp8() if FP8_QKV else bf
    cos_t = np.ascontiguousarray(np.tile(rope_cos.T, (4, 1))).astype(bf)
    # row-permuted signed sin: row p holds sgn(partner)*sin[p%32] where
    # partner = p+-32 within each 64-block, so the shifted rot-multiply
    # reads q_sb and sin at the SAME base partition.  qs[d] = -q[d+32]*sin[d]
    # for d%64<32 (else +q[d-32]*sin[d]) => sin_t[p] = -sin[p%32] for
    # p%64>=32, +sin[p%32] otherwise.
    sgn = np.where((np.arange(128) % 64) < 32, 1.0, -1.0)[:, None]
    sin_t = np.ascontiguousarray(
        np.tile(rope_sin.T, (4, 1)) * sgn).astype(bf)
    tri = np.triu(np.ones((128, 128), np.float32)).astype(bf)
    sub4 = np.tile(subln_w.astype(np.float32), 4)[:, None]

    in_maps = []
    for c in range(8):
        b, g = c // 4, c % 4
        xtc = np.ascontiguousarray(x[b].T).astype(in_t)
        cols = []
        for j in range(NHL):
            h = 4 * g + j
            cols.append(Wq[:, h * 64:(h + 1) * 64])
            cols.append(Wq[:, (H + h) * 64:(H + h + 1) * 64])
        wq_c = np.ascontiguousarray(np.concatenate(cols, axis=1)).astype(in_t)
        wk_c = np.ascontiguousarray(np.concatenate(
            [Wk[:, g * 64:(g + 1) * 64], Wk[:, (KV + g) * 64:(KV + g + 1) * 64]],
            axis=1)).astype(in_t)
        wv_c = np.ascontiguousarray(Wv[:, g * 64:(g + 1) * 64]).astype(in_t)
        wo_c = np.ascontiguousarray(
            Wo[g * 256:(g + 1) * 256, :] * sub4).astype(bf)
        in_maps.append({
            "xt": xtc, "wq": wq_c, "wk": wk_c, "wv": wv_c, "wo": wo_c,
            "perm": _perm_mat().astype(bf),
            "cos_t": cos_t, "sin_t": sin_t, "trimask": tri,
        })
    return in_maps


def _compute_lam(lambda_q1, lambda_k1, lambda_q2, lambda_k2):
    li = 0.8 - 0.6 * math.exp(-0.3)
    l1 = np.exp(np.dot(lambda_q1.astype(np.float32), lambda_k1.astype(np.float32)))
    l2 = np.exp(np.dot(lambda_q2.astype(np.float32), lambda_k2.astype(np.float32)))
    return float(l1 - l2 + li)


def _numpy_reference(x, rope_cos, rope_sin, attention_mask, Wq, Wk, Wv, Wo,
                     lambda_q1, lambda_k1, lambda_q2, lambda_k2, subln_w):
    """Pure-numpy fallback, only used if the mask is not the expected causal one."""
    bsz, seq_len, _ = x.shape

    def rope(t):
        c = np.concatenate([rope_cos, rope_cos], axis=-1)[None, None]
        s = np.concatenate([rope_sin, rope_sin], axis=-1)[None, None]
        t1, t2 = np.split(t, 2, axis=-1)
        rot = np.concatenate([-t2, t1], axis=-1)
        return t * c + rot * s

    q = (x @ Wq).reshape(bsz, seq_len, 2 * H, D)
    q1 = np.transpose(q[:, :, :H], (0, 2, 1, 3))
    q2 = np.transpose(q[:, :, H:], (0, 2, 1, 3))
    k = (x @ Wk).reshape(bsz, seq_len, 2 * KV, D)
    k1 = np.transpose(k[:, :, :KV], (0, 2, 1, 3))
    k2 = np.transpose(k[:, :, KV:], (0, 2, 1, 3))
    v = np.transpose((x @ Wv).reshape(bsz, seq_len, KV, D), (0, 2, 1, 3))
    q1, q2, k1, k2 = rope(q1), rope(q2), rope(k1), rope(k2)
    gr = H // KV
    k1 = np.repeat(k1, gr, axis=1)
    k2 = np.repeat(k2, gr, axis=1)
    v = np.repeat(v, gr, axis=1)
    scale = 1.0 / math.sqrt(D)

    def smax(a):
        a = a - a.max(axis=-1, keepdims=True)
        e = np.exp(a)
        return e / e.sum(axis=-1, keepdims=True)

    a1 = smax(np.einsum("bhqd,bhkd->bhqk", q1, k1) * scale + attention_mask)
    a2 = smax(np.einsum("bhqd,bhkd->bhqk", q2, k2) * scale + attention_mask)
    lam = _compute_lam(lambda_q1, lambda_k1, lambda_q2, lambda_k2)
    attn = a1 - lam * a2
    out = np.einsum("bhqk,bhkd->bhqd", attn, v)
    inv = 1.0 / np.sqrt(np.mean(out * out, axis=-1, keepdims=True) + EPS)
    out = out * inv * subln_w
    out = np.transpose(out, (0, 2, 1, 3)).reshape(bsz, seq_len, HS)
    return (out @ Wo).astype(np.float32)


LAST_RESULT = None


def kernel(x, rope_cos, rope_sin, attention_mask, Wq, Wk, Wv, Wo,
           lambda_q1, lambda_k1, lambda_q2, lambda_k2, subln_w):
    global LAST_RESULT
    x = np.asarray(x, np.float32)
    kk, qq = np.arange(S)[:, None], np.arange(S)[None, :]
    causal = np.where(qq <= kk, 0.0, NEG).astype(np.float32)[None, None]
    am = np.asarray(attention_mask, np.float32)
    if am.shape != (1, 1, S, S) or not np.array_equal(am, causal):
        return _numpy_reference(x, rope_cos, rope_sin, am, Wq, Wk, Wv, Wo,
                                lambda_q1, lambda_k1, lambda_q2, lambda_k2,
                                subln_w)

    lam = _compute_lam(lambda_q1, lambda_k1, lambda_q2, lambda_k2)
    nc = get_program(lam)
    in_maps = _host_inputs(x, np.asarray(rope_cos, np.float32),
                           np.asarray(rope_sin, np.float32),
                           np.asarray(Wq, np.float32), np.asarray(Wk, np.float32),
                           np.asarray(Wv, np.float32), np.asarray(Wo, np.float32),
                           np.asarray(subln_w, np.float32), lam)
    res = bass_utils.run_bass_kernel_spmd(nc, in_maps, core_ids=list(range(8)))
    LAST_RESULT = res
    y = np.zeros((B, S, HS), np.float32)
    for c in range(8):
        y[c // 4] += res.results[c]["out_pt"].T.astype(np.float32)
    return y


# revision 44
# speedup vs baseline: 1.0585x; 1.0585x over previous
"""Differential attention (B=2, S=2048, HS=1024, H=16, KV=4, D=64) on 8 trn2 cores.

Sharding: core c = (b, g) with b = c // 4 (data parallel on batch) and
g = c % 4 (tensor parallel over the 4 KV head groups; each core owns the
4 query heads of its group).  Each core computes its 4 heads' normed
attention output and a row-parallel partial of the output projection
(out_pt = (O_heads @ Wo_rows)^T, bf16); the host upcasts and sums the 4
partials per batch.

QKV projections run in fp8e4(DoubleRow): x and Wq/Wk/Wv are quantized to
e4m3 and hs-chunk PAIRS are contracted per instruction (0.5 cycles/row
at doubled contraction = 4x over bf16).  S and U matmuls stay bf16: that
keeps the PE queue about as busy as the ACT engine (the exp stream is
the intrinsic floor), which matters because an idle PE loses its p-state
ramp.

RoPE without the permutation matmul: rot(q)[d] = ∓q[(d±32) mod 64-block]
is four 32-partition-shifted DVE multiplies against a sign-folded sin
table (32-multiple partition shifts are legal on DVE); q*cos and the
final add run on the otherwise idle Pool engine.

attention(qt), per head: flash-style causal attention over k tiles,
S^T[k,q] strips via two 64-contraction matmuls, P = exp(S/8) on ACT
(no row-max: scores are O(5); diagonal strips exp both branch segments
with ONE strided-AP activation), STAGE=8 k-tiles of S/exp ahead of the
U matmuls, causal wedge zeroed by one dual-block mask-multiply on Pool,
U^T[128,q] += [V|ones].T @ P — the ones block replicates the softmax
denominators onto partitions 64..127.  Epilogue needs one reciprocal
instead of two: out is rms-normalized, so O~ = U1 - lam~*U2 with
lam~ = lam*d1/d2 (a per-query row vector built on the 64 redundant
denominator partitions with reciprocal + scalar_tensor_tensor) has the
same normalized value as O = U1/d1 - lam*U2/d2.

rms: each head's ssq row-sum lands on partition 32j of a [128, QTW]
tile, so one Ln and one Exp over partitions 0..96 (ACT cost is free-size
only) replace per-head activations; Ln/Exp share the preloaded softmax
act table.  Pool also does the rms broadcasts and the onq multiplies.

Pipeline: proj(qt+1) units fill attention(qt); rms(qt-1) at head slot 1,
wo(qt-1) halves at head slots 2/3.  PSUM: psS pairs [128,1024]
double-buffered (4 banks) + psU [128,1024] single (2) + aux ring (2).
"""

import math
import os
import sys

import numpy as np

try:
    import concourse.bass as bass  # noqa: F401
except ImportError:
    sys.path.insert(0, "/opt/trn_rl_repo")

import concourse.bass as bass
import concourse.tile as tile
from concourse import bacc, mybir
from concourse import bass_utils

f32 = mybir.dt.float32
bf16 = mybir.dt.bfloat16
fp8 = mybir.dt.float8e4
AF = mybir.ActivationFunctionType
ALU = mybir.AluOpType
DR = mybir.MatmulPerfMode.DoubleRow

B, S, HS = 2, 2048, 1024
H, KV, D = 16, 4, 64
NHL = 4            # query heads per core
NQT = 4            # q tiles of 512
QTW = 512
NKT = 16           # k tiles of 128
NHS = 8            # hs tiles of 128
NEG = -1e9
EPS = 1e-5

_prog_cache = {}
PHASE_LOG = []
_F = os.environ.get
FP8_QKV = _F("K_FP8_QKV", "0") == "1"
SIMPLE_IN = _F("K_SIMPLE_IN", "0") == "1"
POOL_OFF = _F("K_POOL_OFF", "0") == "1"   # 1: everything back on DVE
DEBUG_DUMP = _F("K_DEBUG_DUMP", "0") == "1"
LAMT_EPI = _F("K_LAMT_EPI", "0") == "1"   # single-recip lam~ epilogue
                                          # (breaks the r1/r2 error
                                          # correlation; adds ~3% err)
OLD_RMS = _F("K_OLD_RMS", "0") == "1"     # per-head [1,W] Ln/Exp
OLD_DIAG = _F("K_OLD_DIAG", "0") == "1"   # two-instr diag exp
OLD_ROPE = _F("K_OLD_ROPE", "0") == "1"   # perm-matmul rot
DUMP_LITE = _F("K_DUMP_LITE", "0") == "1"


def _build_program(lam: float):
    nc = bacc.Bacc("TRN2", target_bir_lowering=False, debug=False,
                   enable_asserts=False, num_devices=8)
    PHASE_LOG.clear()

    def mark(label):
        PHASE_LOG.append((label, nc.next_id()))

    in_dt = fp8 if FP8_QKV else bf16
    xt = nc.dram_tensor("xt", [HS, S], in_dt, kind="ExternalInput").ap()
    wq = nc.dram_tensor("wq", [HS, 512], in_dt, kind="ExternalInput").ap()
    wk = nc.dram_tensor("wk", [HS, 128], in_dt, kind="ExternalInput").ap()
    wv = nc.dram_tensor("wv", [HS, 64], in_dt, kind="ExternalInput").ap()
    wo = nc.dram_tensor("wo", [256, HS], bf16, kind="ExternalInput").ap()
    perm = nc.dram_tensor("perm", [128, 128], bf16, kind="ExternalInput").ap()
    cos_t = nc.dram_tensor("cos_t", [128, S], bf16, kind="ExternalInput").ap()
    sin_t = nc.dram_tensor("sin_t", [128, S], bf16, kind="ExternalInput").ap()
    trimask = nc.dram_tensor("trimask", [128, 128], bf16,
                             kind="ExternalInput").ap()
    out_pt = nc.dram_tensor("out_pt", [HS, S], bf16, kind="ExternalOutput").ap()
    dbg = {}
    if DUMP_LITE:
        for nm, shp in (("dbg_ssqr0", [128, 512]), ("dbg_rmq0", [128, 512]),
                        ("dbg_on0", [128, 512]), ("dbg_on1", [128, 512])):
            dbg[nm] = nc.dram_tensor(nm, shp, f32, kind="ExternalOutput").ap()
    if DEBUG_DUMP:
        for nm, shp in (("dbg_k", [128, S]), ("dbg_q00", [128, 512]),
                        ("dbg_op0", [128, 512]), ("dbg_on0", [128, 512]),
                        ("dbg_ssqr0", [128, 512]), ("dbg_psu00", [128, 1024]),
                        ("dbg_va0", [128, 128])):
            dbg[nm] = nc.dram_tensor(nm, shp, f32, kind="ExternalOutput").ap()

    pool_eng = nc.vector if POOL_OFF else nc.gpsimd

    with tile.TileContext(nc) as tc:
        with tc.tile_pool(name="persist", bufs=1) as pp, \
             tc.tile_pool(name="loc", bufs=3) as loc, \
             tc.tile_pool(name="pwk", bufs=3) as pwk, \
             tc.tile_pool(name="patt", bufs=16) as pa, \
             tc.tile_pool(name="ep", bufs=4) as pe, \
             tc.tile_pool(name="rmsp", bufs=2) as prm, \
             tc.psum_pool(name="ps", bufs=2) as ps_:

            # preload the act-func set that holds BOTH Exp and Ln so the
            # table-load pass never has to switch sets mid-stream
            from concourse.hw_specs import get_activation_tables
            _tables = list(get_activation_tables(nc.m.arch).items())
            _set_id = next(i for i, (_, fs) in enumerate(_tables)
                           if AF.Exp in fs and AF.Ln in fs)
            _ld = mybir.InstLoadActFuncSet(
                name=nc.get_next_instruction_name(),
                act_func_set_id=_set_id, ins=[], outs=[])
            nc.scalar.add_instruction(_ld)

            W = {}
            warm = pp.tile([64, 64], bf16, name="warm", tag="warm")
            nc.vector.memset(warm[:], 1.0)

            def emit_warm(n, dep=None):
                # tiny keep-alive matmuls: hold the PE p-state ramp through
                # windows where no real matmul is ready; `dep` staggers the
                # batch behind a chain-produced tile
                psd = ps_.tile([128, 2 * QTW], f32, name="psd", tag="psS")
                lhs = warm[0:64, 0:1] if dep is None else dep[:, 0:1]
                rhs = warm[0:64, 0:64] if dep is None else dep[:, 0:64]
                for _ in range(n):
                    nc.tensor.matmul(psd[0:1, 0:64], lhs, rhs,
                                     start=True, stop=True,
                                     skip_group_check=True)

            def emit_weight_loads0(state):
                W['wk_a'] = pp.tile([128, 8 * 128], in_dt, name="wk", tag="wk")
                if SIMPLE_IN:
                    for hs in range(NHS):
                        nc.sync.dma_start(
                            W['wk_a'][:, hs * 128:(hs + 1) * 128],
                            wk[hs * 128:(hs + 1) * 128, :])
                else:
                    nc.sync.dma_start(
                        W['wk_a'][:].rearrange("p (h c) -> p h c", h=NHS),
                        wk[:].rearrange("(h p) c -> p h c", h=NHS))
                emit_xt_loads(0, state)
                W['perm_sb'] = pp.tile([128, 128], bf16, name="perm",
                                       tag="perm")
                nc.scalar.dma_start(W['perm_sb'][:], perm[:])
                W['cos_sb'] = pp.tile([128, S], bf16, name="cos", tag="cos")
                nc.scalar.dma_start(W['cos_sb'][:], cos_t[:])
                W['sin_sb'] = pp.tile([128, S], bf16, name="sin", tag="sin")
                nc.sync.dma_start(W['sin_sb'][:], sin_t[:])
                W['wq_a'] = pp.tile([128, 8 * 512], in_dt, name="wq", tag="wq")
                if SIMPLE_IN:
                    for hs in range(NHS):
                        nc.sync.dma_start(
                            W['wq_a'][:, hs * 512:(hs + 1) * 512],
                            wq[hs * 128:(hs + 1) * 128, :])
                else:
                    for t in range(2):
                        nc.sync.dma_start(
                            W['wq_a'][:, t * 2048:(t + 1) * 2048].rearrange(
                                "p (h c) -> p h c", h=4),
                            wq[t * 512:(t + 1) * 512, :].rearrange(
                                "(h p) c -> p h c", h=4))
                W['wv_a'] = pp.tile([128, 8 * 64], in_dt, name="wv", tag="wv")
                if SIMPLE_IN:
                    for hs in range(NHS):
                        nc.scalar.dma_start(
                            W['wv_a'][:, hs * 64:(hs + 1) * 64],
                            wv[hs * 128:(hs + 1) * 128, :])
                else:
                    nc.scalar.dma_start(
                        W['wv_a'][:].rearrange("p (h c) -> p h c", h=NHS),
                        wv[:].rearrange("(h p) c -> p h c", h=NHS))
                W['ones_sb'] = pp.tile([128, 64], bf16, name="ones", tag="ones")
                nc.vector.memset(W['ones_sb'][:], 1.0)
                W['tri_sb'] = pp.tile([128, 128], bf16, name="tri", tag="tri")
                nc.sync.dma_start(W['tri_sb'][:], trimask[:])
                W['wo_sb'] = []
                for t in range(2):
                    w = pp.tile([128, HS], bf16, name=f"wo{t}", tag=f"wo{t}")
                    nc.scalar.dma_start(w[:], wo[t * 128:(t + 1) * 128, :])
                    W['wo_sb'].append(w)
            k_sb = pp.tile([128, S], bf16, name="k", tag="k")
            va = [pp.tile([128, 128], bf16, name=f"va{kt}", tag=f"va{kt}")
                  for kt in range(NKT)]
            for kt in range(NKT):
                nc.vector.memset(va[kt][:, 64:128], 1.0)
            eps_sb = pp.tile([128, 1], f32, name="eps", tag="eps")
            nc.vector.memset(eps_sb[:], EPS)

            def rope_block(ps, dst, qlo, qhi, dst_sb=None):
                # dst = q*cos + rot(q)*sin; rot = 4 partition-32-shifted
                # multiplies.  Both SBUF inputs of a TensorTensor must share
                # their base partition, so the sin table is row-permuted on
                # the host (sin_sb[p] holds the signed sin of the PARTNER row
                # p+-32); qc/add run on Pool
                q_sb = dst_sb
                if q_sb is None:
                    q_sb = pwk.tile([128, QTW], bf16, name="qsb", tag="qsb")
                nc.vector.tensor_copy(q_sb[:], ps[:])
                qc = pwk.tile([128, QTW], bf16, name="qc", tag="qc")
                pool_eng.tensor_mul(qc[:], q_sb[:], W['cos_sb'][:, qlo:qhi])
                if OLD_ROPE:
                    psr = ps_.tile([128, QTW], f32, name="psr", tag="aux")
                    nc.tensor.matmul(psr[:], W['perm_sb'][:], q_sb[:],
                                     start=True, stop=True)
                    qs0 = pwk.tile([128, QTW], bf16, name="qs", tag="qs")
                    nc.vector.tensor_mul(qs0[:], psr[:],
                                         W['sin_sb'][:, qlo:qhi])
                    pool_eng.tensor_add(dst, qc[:], qs0[:])
                    return
                qs = pwk.tile([128, QTW], bf16, name="qs", tag="qs")
                for blk in range(2):
                    b0 = 64 * blk
                    nc.vector.tensor_mul(qs[b0:b0 + 32, :],
                                         q_sb[b0 + 32:b0 + 64, :],
                                         W['sin_sb'][b0 + 32:b0 + 64,
                                                     qlo:qhi])
                    nc.vector.tensor_mul(qs[b0 + 32:b0 + 64, :],
                                         q_sb[b0:b0 + 32, :],
                                         W['sin_sb'][b0:b0 + 32, qlo:qhi])
                pool_eng.tensor_add(dst, qc[:], qs[:])

            def emit_xt_loads(qt, state):
                mark(f"xt{qt}")
                qlo, qhi = qt * QTW, (qt + 1) * QTW
                xt_a = pwk.tile([128, 8 * QTW], in_dt, name="xt", tag="xt")
                if SIMPLE_IN:
                    for hs in range(NHS):
                        eng = nc.scalar if hs % 2 == 0 else nc.sync
                        eng.dma_start(xt_a[:, hs * QTW:(hs + 1) * QTW],
                                      xt[hs * 128:(hs + 1) * 128, qlo:qhi])
                else:
                    for t in range(2):
                        eng = nc.scalar if t == 0 else nc.sync
                        eng.dma_start(
                            xt_a[:, t * 2048:(t + 1) * 2048].rearrange(
                                "p (h c) -> p h c", h=4),
                            xt[t * 512:(t + 1) * 512, qlo:qhi].rearrange(
                                "(h p) c -> p h c", h=4))
                state[(qt, "xt")] = xt_a

            def proj_units(qt, state):
                mark(f"proj{qt}")
                qlo, qhi = qt * QTW, (qt + 1) * QTW
                xt_a = state[(qt, "xt")]
                psk = ps_.tile([128, QTW], f32, name="psk", tag="aux")
                if FP8_QKV:
                    xt_c = xt_a[:].rearrange("p (c n) -> p c n", c=NHS)
                    wk_c = W['wk_a'][:].rearrange("p (c n) -> p c n", c=NHS)
                    wq_c = W['wq_a'][:].rearrange("p (c n) -> p c n", c=NHS)
                    wv_c = W['wv_a'][:].rearrange("p (c n) -> p c n", c=NHS)
                    for u in range(4):
                        nc.tensor.matmul(
                            psk[:], wk_c[:, 2 * u:2 * u + 2, :],
                            xt_c[:, 2 * u:2 * u + 2, :],
                            start=(u == 0), stop=(u == 3), perf_mode=DR)
                        if u % 2 == 1:
                            yield
                else:
                    for half_ in range(2):
                        for hs in range(4 * half_, 4 * half_ + 4):
                            nc.tensor.matmul(
                                psk[:], W['wk_a'][:, hs * 128:(hs + 1) * 128],
                                xt_a[:, hs * 512:(hs + 1) * 512],
                                start=(hs == 0), stop=(hs == NHS - 1))
                        yield
                rope_block(psk, k_sb[:, qlo:qhi], qlo, qhi)
                yield
                qloc = [loc.tile([128, QTW], bf16, name=f"q{jj}", tag=f"q{jj}")
                        for jj in range(NHL)]
                state[qt] = qloc
                for j in range(NHL):
                    psq = ps_.tile([128, QTW], f32, name="psq", tag="aux")
                    if FP8_QKV:
                        for u in range(4):
                            nc.tensor.matmul(
                                psq[:],
                                wq_c[:, 2 * u:2 * u + 2,
                                     j * 128:(j + 1) * 128],
                                xt_c[:, 2 * u:2 * u + 2, :],
                                start=(u == 0), stop=(u == 3), perf_mode=DR)
                            if u % 2 == 1:
                                yield
                    else:
                        for half_ in range(2):
                            for hs in range(4 * half_, 4 * half_ + 4):
                                nc.tensor.matmul(
                                    psq[:],
                                    W['wq_a'][:, hs * 512 + j * 128:
                                              hs * 512 + (j + 1) * 128],
                                    xt_a[:, hs * 512:(hs + 1) * 512],
                                    start=(hs == 0), stop=(hs == NHS - 1))
                            yield
                    rope_block(psq, qloc[j][:], qlo, qhi)
                    yield
                for kk in range(4):
                    kt = 4 * qt + kk
                    psvt = ps_.tile([128, 64], f32, name="psvt", tag="aux")
                    if FP8_QKV:
                        for u in range(4):
                            nc.tensor.matmul(
                                psvt[:],
                                xt_c[:, 2 * u:2 * u + 2,
                                     kk * 128:(kk + 1) * 128],
                                wv_c[:, 2 * u:2 * u + 2, :],
                                start=(u == 0), stop=(u == 3), perf_mode=DR)
                    else:
                        for hs in range(NHS):
                            nc.tensor.matmul(
                                psvt[:],
                                xt_a[:, hs * 512 + kk * 128:
                                     hs * 512 + (kk + 1) * 128],
                                W['wv_a'][:, hs * 64:(hs + 1) * 64],
                                start=(hs == 0), stop=(hs == NHS - 1))
                    nc.vector.tensor_copy(va[kt][:, 0:64], psvt[:])
                    yield

            def emit_att_head(qt, j, state, filler=None):
                mark(f"att{qt}.{j}")
                qloc = state[qt]
                if j == 0:
                    state[(qt, "op")] = [loc.tile([128, QTW], bf16,
                                                  name=f"op{t}", tag=f"op{t}")
                                         for t in range(2)]
                    state[(qt, "on")] = [loc.tile([128, QTW], bf16,
                                                  name=f"on{t}", tag=f"on{t}")
                                         for t in range(2)]
                opair = state[(qt, "op")]
                half, pt = (j % 2) * 64, j // 2
                last_kt = 4 * qt + 3
                psu = ps_.tile([128, 2 * QTW], f32, name="psu", tag="psU",
                               bufs=1)
                p12s = {}

                def emit_s_exp(kt):
                    jd = kt - 4 * qt
                    q0 = 128 * jd if jd >= 0 else 0
                    pss = ps_.tile([128, 2 * QTW], f32, name="pss", tag="psS")
                    nc.tensor.matmul(
                        pss[:, q0:QTW],
                        k_sb[0:64, kt * 128:(kt + 1) * 128],
                        qloc[j][0:64, q0:QTW],
                        start=True, stop=True, skip_group_check=True)
                    nc.tensor.matmul(
                        pss[:, QTW + q0:2 * QTW],
                        k_sb[64:128, kt * 128:(kt + 1) * 128],
                        qloc[j][64:128, q0:QTW],
                        start=True, stop=True, skip_group_check=True)
                    p12 = pa.tile([128, 2 * QTW], bf16, name="p12", tag="p12")
                    if q0 == 0:
                        nc.scalar.activation(p12[:, 0:2 * QTW],
                                             pss[:, 0:2 * QTW],
                                             AF.Exp, scale=0.125)
                    elif OLD_DIAG:
                        nc.scalar.activation(p12[:, q0:QTW], pss[:, q0:QTW],
                                             AF.Exp, scale=0.125)
                        nc.scalar.activation(p12[:, QTW + q0:2 * QTW],
                                             pss[:, QTW + q0:2 * QTW],
                                             AF.Exp, scale=0.125)
                    else:
                        nc.scalar.activation(
                            p12[:].rearrange("p (b q) -> p b q",
                                             b=2)[:, :, q0:QTW],
                            pss[:].rearrange("p (b q) -> p b q",
                                             b=2)[:, :, q0:QTW],
                            AF.Exp, scale=0.125)
                    if jd >= 0:
                        wap = p12[:].rearrange("p (b q) -> p b q",
                                               b=2)[:, :, q0:q0 + 128]
                        msk = W['tri_sb'][:].unsqueeze(1).broadcast_to(
                            [128, 2, 128])
                        pool_eng.tensor_mul(wap, wap, msk)
                    p12s[kt] = p12

                def drain_ssq():
                    if state.get("pend_ssq") is None:
                        return
                    osq_, ssqr_, half_, j_ = state.pop("pend_ssq")
                    psss = ps_.tile([1, QTW], f32, name="psss", tag="aux")
                    nc.tensor.matmul(psss[:],
                                     W['ones_sb'][half_:half_ + 64, 0:1],
                                     osq_[half_:half_ + 64, :],
                                     start=True, stop=True,
                                     skip_group_check=True)
                    nc.vector.tensor_copy(
                        ssqr_[32 * j_:32 * j_ + 1, 0:QTW], psss[0:1, :])

                STAGE = 8
                for kt in range(min(STAGE, last_kt + 1)):
                    emit_s_exp(kt)
                for kt in range(last_kt + 1):
                    if kt == min(3, last_kt):
                        # drain the previous head's ssq row-sum here: by now
                        # its DVE epilogue chain has certainly produced osq,
                        # so the matmul doesn't park the in-order PE queue
                        drain_ssq()
                    if kt + STAGE <= last_kt:
                        emit_s_exp(kt + STAGE)
                    jd = kt - 4 * qt
                    q0 = 128 * jd if jd >= 0 else 0
                    p12 = p12s.pop(kt)
                    # lam lives in the epilogue now, so both branches share
                    # [V|ones] (a matmul output cannot span PSUM banks, so
                    # the two branch segments stay separate instructions)
                    nc.tensor.matmul(
                        psu[:, q0:QTW], va[kt][:], p12[:, q0:QTW],
                        start=(kt == 0), stop=(kt == last_kt),
                        skip_group_check=True)
                    nc.tensor.matmul(
                        psu[:, QTW + q0:2 * QTW], va[kt][:],
                        p12[:, QTW + q0:2 * QTW],
                        start=(kt == 0), stop=(kt == last_kt),
                        skip_group_check=True)

                if filler is not None:
                    for _ in range(5):
                        next(filler, None)
                if DEBUG_DUMP and qt == 0 and j == 0:
                    d_ = pe.tile([128, 1024], f32, name="dpsu", tag="dbgd")
                    nc.vector.tensor_copy(d_[:], psu[:])
                    nc.sync.dma_start(dbg["dbg_psu00"][:], d_[:])
                    for nm, t_ in (("dbg_k", k_sb), ("dbg_q00", qloc[0]),
                                   ("dbg_va0", va[0])):
                        p_, f_ = t_[:].shape
                        d2 = pe.tile([128, S], f32, name=f"d{nm}", tag="dbgd2",
                                     bufs=3)
                        nc.vector.tensor_copy(d2[0:p_, 0:f_], t_[:])
                        nc.sync.dma_start(dbg[nm][:, 0:f_], d2[0:p_, 0:f_])
                mark(f"epi{qt}.{j}")
                if LAMT_EPI:
                    # single-reciprocal O~ = U1 - (lam*d1/d2)*U2; the d1
                    # scale cancels in the rms.  Numerically worse: the two
                    # branches' reciprocal errors no longer cancel in the
                    # differential subtraction.
                    rec2 = pe.tile([64, QTW], f32, name="rec2", tag="rec2")
                    nc.vector.reciprocal(rec2[:], psu[64:128, QTW:2 * QTW])
                    lamt = pe.tile([64, QTW], f32, name="lamt", tag="lamt")
                    nc.vector.scalar_tensor_tensor(
                        lamt[:], rec2[:], lam, psu[64:128, 0:QTW],
                        ALU.mult, ALU.mult)
                    t2 = pe.tile([64, QTW], bf16, name="t2", tag="t2")
                    nc.vector.tensor_mul(t2[:], psu[0:64, QTW:2 * QTW],
                                         lamt[:])
                    nc.vector.tensor_sub(opair[pt][half:half + 64, :],
                                         psu[0:64, 0:QTW], t2[:])
                else:
                    # O = U1/d1 - lam*U2/d2: both reciprocals come from the
                    # same DVE approx, so their errors track (d1 ~= d2) and
                    # cancel in the differential subtraction
                    wri = pe.tile([64, 2 * QTW], f32, name="wri", tag="wri")
                    nc.vector.reciprocal(wri[:], psu[64:128, :])
                    t1 = pe.tile([64, QTW], bf16, name="t1", tag="t1")
                    nc.vector.tensor_mul(t1[:], psu[0:64, 0:QTW],
                                         wri[0:64, 0:QTW])
                    t2 = pe.tile([64, QTW], bf16, name="t2", tag="t2")
                    nc.vector.scalar_tensor_tensor(
                        t2[:], psu[0:64, QTW:2 * QTW], lam,
                        wri[0:64, QTW:2 * QTW], ALU.mult, ALU.mult)
                    nc.vector.tensor_sub(opair[pt][half:half + 64, :],
                                         t1[:], t2[:])
                # fused ssq contribution for the rms: row-sum of O~^2
                if j == 0:
                    ssqr = prm.tile([128, QTW], f32, name="ssqr", tag="ssqr")
                    pool_eng.memset(ssqr[:], 1.0)
                    state[(qt, "ssqr")] = ssqr
                ssqr = state[(qt, "ssqr")]
                osq = prm.tile([128, QTW], bf16, name="osq", tag="osq")
                nc.vector.tensor_mul(osq[half:half + 64, :],
                                     opair[pt][half:half + 64, :],
                                     opair[pt][half:half + 64, :])
                state["pend_ssq"] = (osq, ssqr, half, j)

            def emit_rms(qt, state, pts=(0, 1)):
                mark(f"rms{qt}")
                if state.get("pend_ssq") is not None:
                    osq_, ssqr_, half_, j_ = state.pop("pend_ssq")
                    psss = ps_.tile([1, QTW], f32, name="psss", tag="aux")
                    nc.tensor.matmul(psss[:],
                                     W['ones_sb'][half_:half_ + 64, 0:1],
                                     osq_[half_:half_ + 64, :],
                                     start=True, stop=True,
                                     skip_group_check=True)
                    nc.vector.tensor_copy(
                        ssqr_[32 * j_:32 * j_ + 1, 0:QTW], psss[0:1, :])
                opair = state[(qt, "op")]
                onq = state[(qt, "on")]
                ssqr = state[(qt, "ssqr")]
                # head j's ssq sits on partition 32j; one Ln + one Exp span
                # the whole partition range (ACT cost is free-size only; the
                # in-between rows hold memset filler)
                r0 = 0   # device activations start at partition 0
                r1 = 64 * pts[-1] + 33
                sqr = prm.tile([128, QTW], f32, name="sqr", tag="sqr")
                rmq = prm.tile([128, QTW], bf16, name="rmq", tag="rmq")
                if OLD_RMS:
                    for pt in pts:
                        for rr in (64 * pt, 64 * pt + 32):
                            nc.scalar.activation(sqr[rr:rr + 1, :],
                                                 ssqr[rr:rr + 1, :],
                                                 AF.Ln, scale=1.0 / 64.0,
                                                 bias=eps_sb[rr:rr + 1, 0:1])
                            nc.scalar.activation(rmq[rr:rr + 1, :],
                                                 sqr[rr:rr + 1, :],
                                                 AF.Exp, scale=-0.5)
                else:
                    nc.scalar.activation(sqr[r0:r1, :], ssqr[r0:r1, :],
                                         AF.Ln, scale=1.0 / 64.0,
                                         bias=eps_sb[r0:r1, 0:1])
                    nc.scalar.activation(rmq[r0:r1, :], sqr[r0:r1, :],
                                         AF.Exp, scale=-0.5)
                if DEBUG_DUMP and qt == 0:
                    dss = prm.tile([128, QTW], f32, name="dss", tag="dss")
                    nc.vector.tensor_copy(dss[:], ssqr[:])
                    nc.sync.dma_start(dbg["dbg_ssqr0"][:], dss[:])
                for pt in pts:
                    # partition_broadcast only honors a base-0 input on HW:
                    # stage the two factor rows onto partition 0 first
                    stg = prm.tile([1, 2 * QTW], bf16, name="stg", tag="stg",
                                   bufs=2)
                    nc.vector.tensor_copy(stg[0:1, 0:QTW],
                                          rmq[64 * pt:64 * pt + 1, :])
                    nc.vector.tensor_copy(stg[0:1, QTW:2 * QTW],
                                          rmq[64 * pt + 32:64 * pt + 33, :])
                    rsa = prm.tile([128, QTW], bf16, name="rsa", tag="rsa",
                                   bufs=2)
                    nc.gpsimd.partition_broadcast(rsa[:], stg[0:1, 0:QTW])
                    rsb = prm.tile([128, QTW], bf16, name="rsb", tag="rsb",
                                   bufs=2)
                    nc.gpsimd.partition_broadcast(rsb[:],
                                                  stg[0:1, QTW:2 * QTW])
                    # onq multiplies stay on DVE: they gate the wo matmuls
                    nc.vector.tensor_mul(onq[pt][0:64, :], opair[pt][0:64, :],
                                         rsa[0:64, :])
                    nc.vector.tensor_mul(onq[pt][64:128, :],
                                         opair[pt][64:128, :], rsb[64:128, :])
                if DEBUG_DUMP and qt == 0:
                    for nm, t_ in (("dbg_op0", opair[0]), ("dbg_on0", onq[0])):
                        d_ = prm.tile([128, 512], f32, name=f"e{nm}",
                                      tag="dbge", bufs=2)
                        nc.vector.tensor_copy(d_[:], t_[:])
                        nc.sync.dma_start(dbg[nm][:], d_[:])
                if DUMP_LITE and qt == 0:
                    for nm, t_ in (("dbg_ssqr0", ssqr), ("dbg_rmq0", rmq),
                                   ("dbg_on0", onq[0]), ("dbg_on1", onq[1])):
                        d_ = prm.tile([128, 512], f32, name=f"L{nm}",
                                      tag="dbgL", bufs=4)
                        nc.vector.tensor_copy(d_[:], t_[:])
                        nc.sync.dma_start(dbg[nm][:], d_[:])

            def emit_wo(qt, state, half):
                mark(f"wo{qt}.{half}")
                qlo, qhi = qt * QTW, (qt + 1) * QTW
                onq = state[(qt, "on")]
                oc0 = 4 * half
                ow = prm.tile([128, 4 * QTW], bf16, name="ow", tag="ow")
                for i, oc in enumerate(range(oc0, oc0 + 4)):
                    psw = ps_.tile([128, QTW], f32, name="psw", tag="aux")
                    nc.tensor.matmul(
                        psw[:], W['wo_sb'][0][:, oc * 128:(oc + 1) * 128],
                        onq[0][:], start=True, stop=False)
                    nc.tensor.matmul(
                        psw[:], W['wo_sb'][1][:, oc * 128:(oc + 1) * 128],
                        onq[1][:], start=False, stop=True)
                    nc.vector.tensor_copy(ow[:, i * QTW:(i + 1) * QTW],
                                          psw[:])
                nc.sync.dma_start(
                    out_pt[oc0 * 128:(oc0 + 4) * 128,
                           qlo:qhi].rearrange("(b p) c -> p b c", b=4),
                    ow[:].rearrange("p (b c) -> p b c", b=4))

            def emit_wo_tail(qt, state):
                # last q tile: 8 parallel psum accumulators (attention's psS
                # and psU banks are free by now) so all 8 onq[0] matmuls run
                # while the rms of the second head pair is still in flight
                mark(f"wo{qt}.T")
                qlo, qhi = qt * QTW, (qt + 1) * QTW
                onq = state[(qt, "on")]
                psws = []
                bigA = ps_.tile([128, 2 * QTW], f32, name="woTa", tag="psS")
                bigB = ps_.tile([128, 2 * QTW], f32, name="woTb", tag="psS")
                bigU = ps_.tile([128, 2 * QTW], f32, name="woTu", tag="psU",
                                bufs=1)
                for big in (bigA, bigB, bigU):
                    psws.append(big[:, 0:QTW])
                    psws.append(big[:, QTW:2 * QTW])
                for _ in range(2):
                    aux = ps_.tile([128, QTW], f32, name="woTx", tag="aux")
                    psws.append(aux[:])
                for oc in range(8):
                    nc.tensor.matmul(
                        psws[oc], W['wo_sb'][0][:, oc * 128:(oc + 1) * 128],
                        onq[0][:], start=True, stop=False,
                        skip_group_check=True)
                ow = prm.tile([128, 8 * QTW], bf16, name="owT", tag="owT")
                for oc in range(8):
                    nc.tensor.matmul(
                        psws[oc], W['wo_sb'][1][:, oc * 128:(oc + 1) * 128],
                        onq[1][:], start=False, stop=True,
                        skip_group_check=True)
                    nc.vector.tensor_copy(ow[:, oc * QTW:(oc + 1) * QTW],
                                          psws[oc])
                for half in range(2):
                    nc.sync.dma_start(
                        out_pt[half * 512:(half + 1) * 512,
                               qlo:qhi].rearrange("(b p) c -> p b c", b=4),
                        ow[:, half * 2048:(half + 1) * 2048].rearrange(
                            "p (b c) -> p b c", b=4))

            state = {}
            emit_weight_loads0(state)
            for _ in proj_units(0, state):
                pass
            for qt in range(NQT):
                gen = None
                if qt < NQT - 1:
                    emit_xt_loads(qt + 1, state)
                    gen = proj_units(qt + 1, state)
                for j in range(NHL):
                    emit_att_head(qt, j, state, filler=gen)
                    if j == 1 and qt > 0:
                        emit_rms(qt - 1, state)
                    if j == 1 and qt == NQT - 1:
                        emit_rms(qt, state, pts=(0,))
                    if j == 2 and qt > 0:
                        emit_wo(qt - 1, state, half=0)
                    if j == 3 and qt > 0:
                        emit_wo(qt - 1, state, half=1)
                if gen is not None:
                    for _ in gen:
                        pass
            emit_rms(NQT - 1, state, pts=(1,))
            emit_wo_tail(NQT - 1, state)
    nc.compile()
    return nc


def get_program(lam: float):
    key = (round(float(lam), 9), FP8_QKV, SIMPLE_IN, POOL_OFF, DEBUG_DUMP,
           LAMT_EPI, OLD_RMS, OLD_DIAG, OLD_ROPE, DUMP_LITE)
    if key not in _prog_cache:
        _prog_cache[key] = _build_program(float(lam))
    return _prog_cache[key]


def _bf16():
    import ml_dtypes
    return ml_dtypes.bfloat16


def _fp8():
    import ml_dtypes
    return ml_dtypes.float8_e4m3


def _perm_mat():
    # For OLD_ROPE: psr = perm.T @ q with psr[d] = rot(q)[d]/sgn2(d) where
    # sgn2 is the sign folded into sin_t; both cases come out -1.
    p = np.zeros((128, 128), np.float32)
    for o in range(128):
        if o % 64 < 32:
            p[o + 32, o] = -1.0
        else:
            p[o - 32, o] = -1.0
    return p


def _host_inputs(x, rope_cos, rope_sin, Wq, Wk, Wv, Wo, subln_w, lam):
    bf = _bf16()
    in_t = _fp8() if FP8_QKV else bf
    cos_t = np.ascontiguousarray(np.tile(rope_cos.T, (4, 1))).astype(bf)
    # row-permuted signed sin: row p holds sgn(partner)*sin[p%32] where
    # partner = p+-32 within each 64-block, so the shifted rot-multiply
    # reads q_sb and sin at the SAME base partition.  qs[d] = -q[d+32]*sin[d]
    # for d%64<32 (else +q[d-32]*sin[d]) => sin_t[p] = -sin[p%32] for
    # p%64>=32, +sin[p%32] otherwise.
    sgn = np.where((np.arange(128) % 64) < 32, 1.0, -1.0)[:, None]
    sin_t = np.ascontiguousarray(
        np.tile(rope_sin.T, (4, 1)) * sgn).astype(bf)
    tri = np.triu(np.ones((128, 128), np.float32)).astype(bf)
    sub4 = np.tile(subln_w.astype(np.float32), 4)[:, None]

    in_maps = []
    for c in range(8):
        b, g = c // 4, c % 4
        xtc = np.ascontiguousarray(x[b].T).astype(in_t)
        cols = []
        for j in range(NHL):
            h = 4 * g + j
            cols.append(Wq[:, h * 64:(h + 1) * 64])
            cols.append(Wq[:, (H + h) * 64:(H + h + 1) * 64])
        wq_c = np.ascontiguousarray(np.concatenate(cols, axis=1)).astype(in_t)
        wk_c = np.ascontiguousarray(np.concatenate(
            [Wk[:, g * 64:(g + 1) * 64], Wk[:, (KV + g) * 64:(KV + g + 1) * 64]],
            axis=1)).astype(in_t)
        wv_c = np.ascontiguousarray(Wv[:, g * 64:(g + 1) * 64]).astype(in_t)
        wo_c = np.ascontiguousarray(
            Wo[g * 256:(g + 1) * 256, :] * sub4).astype(bf)
        in_maps.append({
            "xt": xtc, "wq": wq_c, "wk": wk_c, "wv": wv_c, "wo": wo_c,
            "perm": _perm_mat().astype(bf),
            "cos_t": cos_t, "sin_t": sin_t, "trimask": tri,
        })
    return in_maps


def _compute_lam(lambda_q1, lambda_k1, lambda_q2, lambda_k2):
    li = 0.8 - 0.6 * math.exp(-0.3)
    l1 = np.exp(np.dot(lambda_q1.astype(np.float32), lambda_k1.astype(np.float32)))
    l2 = np.exp(np.dot(lambda_q2.astype(np.float32), lambda_k2.astype(np.float32)))
    return float(l1 - l2 + li)


def _numpy_reference(x, rope_cos, rope_sin, attention_mask, Wq, Wk, Wv, Wo,
                     lambda_q1, lambda_k1, lambda_q2, lambda_k2, subln_w):
    """Pure-numpy fallback, only used if the mask is not the expected causal one."""
    bsz, seq_len, _ = x.shape

    def rope(t):
        c = np.concatenate([rope_cos, rope_cos], axis=-1)[None, None]
        s = np.concatenate([rope_sin, rope_sin], axis=-1)[None, None]
        t1, t2 = np.split(t, 2, axis=-1)
        rot = np.concatenate([-t2, t1], axis=-1)
        return t * c + rot * s

    q = (x @ Wq).reshape(bsz, seq_len, 2 * H, D)
    q1 = np.transpose(q[:, :, :H], (0, 2, 1, 3))
    q2 = np.transpose(q[:, :, H:], (0, 2, 1, 3))
    k = (x @ Wk).reshape(bsz, seq_len, 2 * KV, D)
    k1 = np.transpose(k[:, :, :KV], (0, 2, 1, 3))
    k2 = np.transpose(k[:, :, KV:], (0, 2, 1, 3))
    v = np.transpose((x @ Wv).reshape(bsz, seq_len, KV, D), (0, 2, 1, 3))
    q1, q2, k1, k2 = rope(q1), rope(q2), rope(k1), rope(k2)
    gr = H // KV
    k1 = np.repeat(k1, gr, axis=1)
    k2 = np.repeat(k2, gr, axis=1)
    v = np.repeat(v, gr, axis=1)
    scale = 1.0 / math.sqrt(D)

    def smax(a):
        a = a - a.max(axis=-1, keepdims=True)
        e = np.exp(a)
        return e / e.sum(axis=-1, keepdims=True)

    a1 = smax(np.einsum("bhqd,bhkd->bhqk", q1, k1) * scale + attention_mask)
    a2 = smax(np.einsum("bhqd,bhkd->bhqk", q2, k2) * scale + attention_mask)
    lam = _compute_lam(lambda_q1, lambda_k1, lambda_q2, lambda_k2)
    attn = a1 - lam * a2
    out = np.einsum("bhqk,bhkd->bhqd", attn, v)
    inv = 1.0 / np.sqrt(np.mean(out * out, axis=-1, keepdims=True) + EPS)
    out = out * inv * subln_w
    out = np.transpose(out, (0, 2, 1, 3)).reshape(bsz, seq_len, HS)
    return (out @ Wo).astype(np.float32)


LAST_RESULT = None


def kernel(x, rope_cos, rope_sin, attention_mask, Wq, Wk, Wv, Wo,
           lambda_q1, lambda_k1, lambda_q2, lambda_k2, subln_w):
    global LAST_RESULT
    x = np.asarray(x, np.float32)
    kk, qq = np.arange(S)[:, None], np.arange(S)[None, :]
    causal = np.where(qq <= kk, 0.0, NEG).astype(np.float32)[None, None]
    am = np.asarray(attention_mask, np.float32)
    if am.shape != (1, 1, S, S) or not np.array_equal(am, causal):
        return _numpy_reference(x, rope_cos, rope_sin, am, Wq, Wk, Wv, Wo,
                                lambda_q1, lambda_k1, lambda_q2, lambda_k2,
                                subln_w)

    lam = _compute_lam(lambda_q1, lambda_k1, lambda_q2, lambda_k2)
    nc = get_program(lam)
    in_maps = _host_inputs(x, np.asarray(rope_cos, np.float32),
                           np.asarray(rope_sin, np.float32),
                           np.asarray(Wq, np.float32), np.asarray(Wk, np.float32),
                           np.asarray(Wv, np.float32), np.asarray(Wo, np.float32),
                           np.asarray(subln_w, np.float32), lam)
    res = bass_utils.run_bass_kernel_spmd(nc, in_maps, core_ids=list(range(8)))
    LAST_RESULT = res
    y = np.zeros((B, S, HS), np.float32)
    for c in range(8):
        y[c // 4] += res.results[c]["out_pt"].T.astype(np.float32)
    return y


# revision 49
# speedup vs baseline: 1.0632x; 1.0044x over previous
"""Differential attention (B=2, S=2048, HS=1024, H=16, KV=4, D=64) on 8 trn2 cores.

Sharding: core c = (b, g) with b = c // 4 (data parallel on batch) and
g = c % 4 (tensor parallel over the 4 KV head groups; each core owns the
4 query heads of its group).  Each core computes its 4 heads' normed
attention output and a row-parallel partial of the output projection
(out_pt = (O_heads @ Wo_rows)^T, bf16); the host upcasts and sums the 4
partials per batch.

QKV projections run in fp8e4(DoubleRow): x and Wq/Wk/Wv are quantized to
e4m3 and hs-chunk PAIRS are contracted per instruction (0.5 cycles/row
at doubled contraction = 4x over bf16).  S and U matmuls stay bf16: that
keeps the PE queue about as busy as the ACT engine (the exp stream is
the intrinsic floor), which matters because an idle PE loses its p-state
ramp.

RoPE without the permutation matmul: rot(q)[d] = ∓q[(d±32) mod 64-block]
is four 32-partition-shifted DVE multiplies against a sign-folded sin
table (32-multiple partition shifts are legal on DVE); q*cos and the
final add run on the otherwise idle Pool engine.

attention(qt), per head: flash-style causal attention over k tiles,
S^T[k,q] strips via two 64-contraction matmuls, P = exp(S/8) on ACT
(no row-max: scores are O(5); diagonal strips exp both branch segments
with ONE strided-AP activation), STAGE=8 k-tiles of S/exp ahead of the
U matmuls, causal wedge zeroed by one dual-block mask-multiply on Pool,
U^T[128,q] += [V|ones].T @ P — the ones block replicates the softmax
denominators onto partitions 64..127.  Epilogue needs one reciprocal
instead of two: out is rms-normalized, so O~ = U1 - lam~*U2 with
lam~ = lam*d1/d2 (a per-query row vector built on the 64 redundant
denominator partitions with reciprocal + scalar_tensor_tensor) has the
same normalized value as O = U1/d1 - lam*U2/d2.

rms: each head's ssq row-sum lands on partition 32j of a [128, QTW]
tile, so one Ln and one Exp over partitions 0..96 (ACT cost is free-size
only) replace per-head activations; Ln/Exp share the preloaded softmax
act table.  Pool also does the rms broadcasts and the onq multiplies.

Pipeline: proj(qt+1) units fill attention(qt); rms(qt-1) at head slot 1,
wo(qt-1) halves at head slots 2/3.  PSUM: psS pairs [128,1024]
double-buffered (4 banks) + psU [128,1024] single (2) + aux ring (2).
"""

import math
import os
import sys

import numpy as np

try:
    import concourse.bass as bass  # noqa: F401
except ImportError:
    sys.path.insert(0, "/opt/trn_rl_repo")

import concourse.bass as bass
import concourse.tile as tile
from concourse import bacc, mybir
from concourse import bass_utils

f32 = mybir.dt.float32
bf16 = mybir.dt.bfloat16
fp8 = mybir.dt.float8e4
AF = mybir.ActivationFunctionType
ALU = mybir.AluOpType
DR = mybir.MatmulPerfMode.DoubleRow

B, S, HS = 2, 2048, 1024
H, KV, D = 16, 4, 64
NHL = 4            # query heads per core
NQT = 4            # q tiles of 512
QTW = 512
NKT = 16           # k tiles of 128
NHS = 8            # hs tiles of 128
NEG = -1e9
EPS = 1e-5

_prog_cache = {}
PHASE_LOG = []
_F = os.environ.get
FP8_QKV = _F("K_FP8_QKV", "0") == "1"
SIMPLE_IN = _F("K_SIMPLE_IN", "0") == "1"
POOL_OFF = _F("K_POOL_OFF", "0") == "1"   # 1: everything back on DVE
DEBUG_DUMP = _F("K_DEBUG_DUMP", "0") == "1"
LAMT_EPI = _F("K_LAMT_EPI", "0") == "1"   # single-recip lam~ epilogue
                                          # (breaks the r1/r2 error
                                          # correlation; adds ~3% err)
OLD_RMS = _F("K_OLD_RMS", "0") == "1"     # per-head [1,W] Ln/Exp
OLD_DIAG = _F("K_OLD_DIAG", "0") == "1"   # two-instr diag exp
OLD_ROPE = _F("K_OLD_ROPE", "0") == "1"   # perm-matmul rot
DUMP_LITE = _F("K_DUMP_LITE", "0") == "1"


def _build_program(lam: float):
    nc = bacc.Bacc("TRN2", target_bir_lowering=False, debug=False,
                   enable_asserts=False, num_devices=8)
    PHASE_LOG.clear()

    def mark(label):
        PHASE_LOG.append((label, nc.next_id()))

    in_dt = fp8 if FP8_QKV else bf16
    xt = nc.dram_tensor("xt", [HS, S], in_dt, kind="ExternalInput").ap()
    wq = nc.dram_tensor("wq", [HS, 512], in_dt, kind="ExternalInput").ap()
    wk = nc.dram_tensor("wk", [HS, 128], in_dt, kind="ExternalInput").ap()
    wv = nc.dram_tensor("wv", [HS, 64], in_dt, kind="ExternalInput").ap()
    wo = nc.dram_tensor("wo", [256, HS], bf16, kind="ExternalInput").ap()
    perm = nc.dram_tensor("perm", [128, 128], bf16, kind="ExternalInput").ap()
    cos_t = nc.dram_tensor("cos_t", [128, S], bf16, kind="ExternalInput").ap()
    sin_t = nc.dram_tensor("sin_t", [128, S], bf16, kind="ExternalInput").ap()
    trimask = nc.dram_tensor("trimask", [128, 128], bf16,
                             kind="ExternalInput").ap()
    out_pt = nc.dram_tensor("out_pt", [HS, S], bf16, kind="ExternalOutput").ap()
    dbg = {}
    if DUMP_LITE:
        for nm, shp in (("dbg_ssqr0", [128, 512]), ("dbg_rmq0", [128, 512]),
                        ("dbg_on0", [128, 512]), ("dbg_on1", [128, 512])):
            dbg[nm] = nc.dram_tensor(nm, shp, f32, kind="ExternalOutput").ap()
    if DEBUG_DUMP:
        for nm, shp in (("dbg_k", [128, S]), ("dbg_q00", [128, 512]),
                        ("dbg_op0", [128, 512]), ("dbg_on0", [128, 512]),
                        ("dbg_ssqr0", [128, 512]), ("dbg_psu00", [128, 1024]),
                        ("dbg_va0", [128, 128])):
            dbg[nm] = nc.dram_tensor(nm, shp, f32, kind="ExternalOutput").ap()

    pool_eng = nc.vector if POOL_OFF else nc.gpsimd

    with tile.TileContext(nc) as tc:
        with tc.tile_pool(name="persist", bufs=1) as pp, \
             tc.tile_pool(name="loc", bufs=3) as loc, \
             tc.tile_pool(name="pwk", bufs=3) as pwk, \
             tc.tile_pool(name="patt", bufs=16) as pa, \
             tc.tile_pool(name="ep", bufs=4) as pe, \
             tc.tile_pool(name="rmsp", bufs=2) as prm, \
             tc.psum_pool(name="ps", bufs=2) as ps_:

            # preload the act-func set that holds BOTH Exp and Ln so the
            # table-load pass never has to switch sets mid-stream
            from concourse.hw_specs import get_activation_tables
            _tables = list(get_activation_tables(nc.m.arch).items())
            _set_id = next(i for i, (_, fs) in enumerate(_tables)
                           if AF.Exp in fs and AF.Ln in fs)
            _ld = mybir.InstLoadActFuncSet(
                name=nc.get_next_instruction_name(),
                act_func_set_id=_set_id, ins=[], outs=[])
            nc.scalar.add_instruction(_ld)

            W = {}
            warm = pp.tile([64, 64], bf16, name="warm", tag="warm")
            nc.vector.memset(warm[:], 1.0)

            def emit_warm(n, dep=None):
                # tiny keep-alive matmuls: hold the PE p-state ramp through
                # windows where no real matmul is ready; `dep` staggers the
                # batch behind a chain-produced tile
                psd = ps_.tile([128, 2 * QTW], f32, name="psd", tag="psS")
                lhs = warm[0:64, 0:1] if dep is None else dep[:, 0:1]
                rhs = warm[0:64, 0:64] if dep is None else dep[:, 0:64]
                for _ in range(n):
                    nc.tensor.matmul(psd[0:1, 0:64], lhs, rhs,
                                     start=True, stop=True,
                                     skip_group_check=True)

            def emit_weight_loads0(state):
                W['wk_a'] = pp.tile([128, 8 * 128], in_dt, name="wk", tag="wk")
                if SIMPLE_IN:
                    for hs in range(NHS):
                        nc.sync.dma_start(
                            W['wk_a'][:, hs * 128:(hs + 1) * 128],
                            wk[hs * 128:(hs + 1) * 128, :])
                else:
                    nc.sync.dma_start(
                        W['wk_a'][:].rearrange("p (h c) -> p h c", h=NHS),
                        wk[:].rearrange("(h p) c -> p h c", h=NHS))
                emit_xt_loads(0, state)
                W['wq_a'] = pp.tile([128, 8 * 512], in_dt, name="wq", tag="wq")
                if SIMPLE_IN:
                    for hs in range(NHS):
                        nc.sync.dma_start(
                            W['wq_a'][:, hs * 512:(hs + 1) * 512],
                            wq[hs * 128:(hs + 1) * 128, :])
                else:
                    for t in range(2):
                        nc.sync.dma_start(
                            W['wq_a'][:, t * 2048:(t + 1) * 2048].rearrange(
                                "p (h c) -> p h c", h=4),
                            wq[t * 512:(t + 1) * 512, :].rearrange(
                                "(h p) c -> p h c", h=4))
                W['wv_a'] = pp.tile([128, 8 * 64], in_dt, name="wv", tag="wv")
                if SIMPLE_IN:
                    for hs in range(NHS):
                        nc.scalar.dma_start(
                            W['wv_a'][:, hs * 64:(hs + 1) * 64],
                            wv[hs * 128:(hs + 1) * 128, :])
                else:
                    nc.scalar.dma_start(
                        W['wv_a'][:].rearrange("p (h c) -> p h c", h=NHS),
                        wv[:].rearrange("(h p) c -> p h c", h=NHS))
                W['perm_sb'] = pp.tile([128, 128], bf16, name="perm",
                                       tag="perm")
                nc.scalar.dma_start(W['perm_sb'][:], perm[:])
                W['cos_sb'] = pp.tile([128, S], bf16, name="cos", tag="cos")
                nc.scalar.dma_start(W['cos_sb'][:], cos_t[:])
                W['sin_sb'] = pp.tile([128, S], bf16, name="sin", tag="sin")
                nc.sync.dma_start(W['sin_sb'][:], sin_t[:])
                W['ones_sb'] = pp.tile([128, 64], bf16, name="ones", tag="ones")
                nc.vector.memset(W['ones_sb'][:], 1.0)
                W['tri_sb'] = pp.tile([128, 128], bf16, name="tri", tag="tri")
                nc.sync.dma_start(W['tri_sb'][:], trimask[:])
                W['wo_sb'] = []
                for t in range(2):
                    w = pp.tile([128, HS], bf16, name=f"wo{t}", tag=f"wo{t}")
                    nc.scalar.dma_start(w[:], wo[t * 128:(t + 1) * 128, :])
                    W['wo_sb'].append(w)
            k_sb = pp.tile([128, S], bf16, name="k", tag="k")
            va = [pp.tile([128, 128], bf16, name=f"va{kt}", tag=f"va{kt}")
                  for kt in range(NKT)]
            for kt in range(NKT):
                nc.vector.memset(va[kt][:, 64:128], 1.0)
            eps_sb = pp.tile([128, 1], f32, name="eps", tag="eps")
            nc.vector.memset(eps_sb[:], EPS)

            def rope_block(ps, dst, qlo, qhi, dst_sb=None):
                # dst = q*cos + rot(q)*sin; rot = 4 partition-32-shifted
                # multiplies.  Both SBUF inputs of a TensorTensor must share
                # their base partition, so the sin table is row-permuted on
                # the host (sin_sb[p] holds the signed sin of the PARTNER row
                # p+-32); qc/add run on Pool
                q_sb = dst_sb
                if q_sb is None:
                    q_sb = pwk.tile([128, QTW], bf16, name="qsb", tag="qsb")
                nc.vector.tensor_copy(q_sb[:], ps[:])
                qc = pwk.tile([128, QTW], bf16, name="qc", tag="qc")
                pool_eng.tensor_mul(qc[:], q_sb[:], W['cos_sb'][:, qlo:qhi])
                if OLD_ROPE:
                    psr = ps_.tile([128, QTW], f32, name="psr", tag="aux")
                    nc.tensor.matmul(psr[:], W['perm_sb'][:], q_sb[:],
                                     start=True, stop=True)
                    qs0 = pwk.tile([128, QTW], bf16, name="qs", tag="qs")
                    nc.vector.tensor_mul(qs0[:], psr[:],
                                         W['sin_sb'][:, qlo:qhi])
                    pool_eng.tensor_add(dst, qc[:], qs0[:])
                    return
                qs = pwk.tile([128, QTW], bf16, name="qs", tag="qs")
                for blk in range(2):
                    b0 = 64 * blk
                    nc.vector.tensor_mul(qs[b0:b0 + 32, :],
                                         q_sb[b0 + 32:b0 + 64, :],
                                         W['sin_sb'][b0 + 32:b0 + 64,
                                                     qlo:qhi])
                    nc.vector.tensor_mul(qs[b0 + 32:b0 + 64, :],
                                         q_sb[b0:b0 + 32, :],
                                         W['sin_sb'][b0:b0 + 32, qlo:qhi])
                pool_eng.tensor_add(dst, qc[:], qs[:])

            def emit_xt_loads(qt, state):
                mark(f"xt{qt}")
                qlo, qhi = qt * QTW, (qt + 1) * QTW
                xt_a = pwk.tile([128, 8 * QTW], in_dt, name="xt", tag="xt")
                if SIMPLE_IN:
                    for hs in range(NHS):
                        eng = nc.scalar if hs % 2 == 0 else nc.sync
                        eng.dma_start(xt_a[:, hs * QTW:(hs + 1) * QTW],
                                      xt[hs * 128:(hs + 1) * 128, qlo:qhi])
                else:
                    for t in range(4):
                        eng = nc.scalar if t % 2 == 0 else nc.sync
                        eng.dma_start(
                            xt_a[:, t * 1024:(t + 1) * 1024].rearrange(
                                "p (h c) -> p h c", h=2),
                            xt[t * 256:(t + 1) * 256, qlo:qhi].rearrange(
                                "(h p) c -> p h c", h=2))
                state[(qt, "xt")] = xt_a

            def proj_units(qt, state):
                mark(f"proj{qt}")
                qlo, qhi = qt * QTW, (qt + 1) * QTW
                xt_a = state[(qt, "xt")]
                psk = ps_.tile([128, QTW], f32, name="psk", tag="aux")
                if FP8_QKV:
                    xt_c = xt_a[:].rearrange("p (c n) -> p c n", c=NHS)
                    wk_c = W['wk_a'][:].rearrange("p (c n) -> p c n", c=NHS)
                    wq_c = W['wq_a'][:].rearrange("p (c n) -> p c n", c=NHS)
                    wv_c = W['wv_a'][:].rearrange("p (c n) -> p c n", c=NHS)
                    for u in range(4):
                        nc.tensor.matmul(
                            psk[:], wk_c[:, 2 * u:2 * u + 2, :],
                            xt_c[:, 2 * u:2 * u + 2, :],
                            start=(u == 0), stop=(u == 3), perf_mode=DR)
                        if u % 2 == 1:
                            yield
                else:
                    for half_ in range(2):
                        for hs in range(4 * half_, 4 * half_ + 4):
                            nc.tensor.matmul(
                                psk[:], W['wk_a'][:, hs * 128:(hs + 1) * 128],
                                xt_a[:, hs * 512:(hs + 1) * 512],
                                start=(hs == 0), stop=(hs == NHS - 1))
                        yield
                rope_block(psk, k_sb[:, qlo:qhi], qlo, qhi)
                yield
                qloc = [loc.tile([128, QTW], bf16, name=f"q{jj}", tag=f"q{jj}")
                        for jj in range(NHL)]
                state[qt] = qloc
                for j in range(NHL):
                    psq = ps_.tile([128, QTW], f32, name="psq", tag="aux")
                    if FP8_QKV:
                        for u in range(4):
                            nc.tensor.matmul(
                                psq[:],
                                wq_c[:, 2 * u:2 * u + 2,
                                     j * 128:(j + 1) * 128],
                                xt_c[:, 2 * u:2 * u + 2, :],
                                start=(u == 0), stop=(u == 3), perf_mode=DR)
                            if u % 2 == 1:
                                yield
                    else:
                        for half_ in range(2):
                            for hs in range(4 * half_, 4 * half_ + 4):
                                nc.tensor.matmul(
                                    psq[:],
                                    W['wq_a'][:, hs * 512 + j * 128:
                                              hs * 512 + (j + 1) * 128],
                                    xt_a[:, hs * 512:(hs + 1) * 512],
                                    start=(hs == 0), stop=(hs == NHS - 1))
                            yield
                    rope_block(psq, qloc[j][:], qlo, qhi)
                    yield
                for kk in range(4):
                    kt = 4 * qt + kk
                    psvt = ps_.tile([128, 64], f32, name="psvt", tag="aux")
                    if FP8_QKV:
                        for u in range(4):
                            nc.tensor.matmul(
                                psvt[:],
                                xt_c[:, 2 * u:2 * u + 2,
                                     kk * 128:(kk + 1) * 128],
                                wv_c[:, 2 * u:2 * u + 2, :],
                                start=(u == 0), stop=(u == 3), perf_mode=DR)
                    else:
                        for hs in range(NHS):
                            nc.tensor.matmul(
                                psvt[:],
                                xt_a[:, hs * 512 + kk * 128:
                                     hs * 512 + (kk + 1) * 128],
                                W['wv_a'][:, hs * 64:(hs + 1) * 64],
                                start=(hs == 0), stop=(hs == NHS - 1))
                    nc.vector.tensor_copy(va[kt][:, 0:64], psvt[:])
                    yield

            def emit_att_head(qt, j, state, filler=None):
                mark(f"att{qt}.{j}")
                qloc = state[qt]
                if j == 0:
                    state[(qt, "op")] = [loc.tile([128, QTW], bf16,
                                                  name=f"op{t}", tag=f"op{t}")
                                         for t in range(2)]
                    state[(qt, "on")] = [loc.tile([128, QTW], bf16,
                                                  name=f"on{t}", tag=f"on{t}")
                                         for t in range(2)]
                opair = state[(qt, "op")]
                half, pt = (j % 2) * 64, j // 2
                last_kt = 4 * qt + 3
                psu = ps_.tile([128, 2 * QTW], f32, name="psu", tag="psU",
                               bufs=1)
                p12s = {}

                def emit_s_exp(kt):
                    jd = kt - 4 * qt
                    q0 = 128 * jd if jd >= 0 else 0
                    pss = ps_.tile([128, 2 * QTW], f32, name="pss", tag="psS")
                    nc.tensor.matmul(
                        pss[:, q0:QTW],
                        k_sb[0:64, kt * 128:(kt + 1) * 128],
                        qloc[j][0:64, q0:QTW],
                        start=True, stop=True, skip_group_check=True)
                    nc.tensor.matmul(
                        pss[:, QTW + q0:2 * QTW],
                        k_sb[64:128, kt * 128:(kt + 1) * 128],
                        qloc[j][64:128, q0:QTW],
                        start=True, stop=True, skip_group_check=True)
                    p12 = pa.tile([128, 2 * QTW], bf16, name="p12", tag="p12")
                    if q0 == 0:
                        nc.scalar.activation(p12[:, 0:2 * QTW],
                                             pss[:, 0:2 * QTW],
                                             AF.Exp, scale=0.125)
                    elif OLD_DIAG:
                        nc.scalar.activation(p12[:, q0:QTW], pss[:, q0:QTW],
                                             AF.Exp, scale=0.125)
                        nc.scalar.activation(p12[:, QTW + q0:2 * QTW],
                                             pss[:, QTW + q0:2 * QTW],
                                             AF.Exp, scale=0.125)
                    else:
                        nc.scalar.activation(
                            p12[:].rearrange("p (b q) -> p b q",
                                             b=2)[:, :, q0:QTW],
                            pss[:].rearrange("p (b q) -> p b q",
                                             b=2)[:, :, q0:QTW],
                            AF.Exp, scale=0.125)
                    if jd >= 0:
                        wap = p12[:].rearrange("p (b q) -> p b q",
                                               b=2)[:, :, q0:q0 + 128]
                        msk = W['tri_sb'][:].unsqueeze(1).broadcast_to(
                            [128, 2, 128])
                        pool_eng.tensor_mul(wap, wap, msk)
                    p12s[kt] = p12

                def drain_ssq():
                    if state.get("pend_ssq") is None:
                        return
                    osq_, ssqr_, half_, j_ = state.pop("pend_ssq")
                    psss = ps_.tile([1, QTW], f32, name="psss", tag="aux")
                    nc.tensor.matmul(psss[:],
                                     W['ones_sb'][half_:half_ + 64, 0:1],
                                     osq_[half_:half_ + 64, :],
                                     start=True, stop=True,
                                     skip_group_check=True)
                    nc.vector.tensor_copy(
                        ssqr_[32 * j_:32 * j_ + 1, 0:QTW], psss[0:1, :])

                STAGE = 8
                for kt in range(min(STAGE, last_kt + 1)):
                    emit_s_exp(kt)
                for kt in range(last_kt + 1):
                    if kt == min(3, last_kt):
                        # drain the previous head's ssq row-sum here: by now
                        # its DVE epilogue chain has certainly produced osq,
                        # so the matmul doesn't park the in-order PE queue
                        drain_ssq()
                    if kt + STAGE <= last_kt:
                        emit_s_exp(kt + STAGE)
                    jd = kt - 4 * qt
                    q0 = 128 * jd if jd >= 0 else 0
                    p12 = p12s.pop(kt)
                    # lam lives in the epilogue now, so both branches share
                    # [V|ones] (a matmul output cannot span PSUM banks, so
                    # the two branch segments stay separate instructions)
                    nc.tensor.matmul(
                        psu[:, q0:QTW], va[kt][:], p12[:, q0:QTW],
                        start=(kt == 0), stop=(kt == last_kt),
                        skip_group_check=True)
                    nc.tensor.matmul(
                        psu[:, QTW + q0:2 * QTW], va[kt][:],
                        p12[:, QTW + q0:2 * QTW],
                        start=(kt == 0), stop=(kt == last_kt),
                        skip_group_check=True)

                if filler is not None:
                    for _ in range(5):
                        next(filler, None)
                if DEBUG_DUMP and qt == 0 and j == 0:
                    d_ = pe.tile([128, 1024], f32, name="dpsu", tag="dbgd")
                    nc.vector.tensor_copy(d_[:], psu[:])
                    nc.sync.dma_start(dbg["dbg_psu00"][:], d_[:])
                    for nm, t_ in (("dbg_k", k_sb), ("dbg_q00", qloc[0]),
                                   ("dbg_va0", va[0])):
                        p_, f_ = t_[:].shape
                        d2 = pe.tile([128, S], f32, name=f"d{nm}", tag="dbgd2",
                                     bufs=3)
                        nc.vector.tensor_copy(d2[0:p_, 0:f_], t_[:])
                        nc.sync.dma_start(dbg[nm][:, 0:f_], d2[0:p_, 0:f_])
                mark(f"epi{qt}.{j}")
                # snapshot psu to SBUF with ONE read so the single psU
                # buffer frees for the next head's first U matmul ~2us
                # earlier than letting the whole epilogue chain read it
                usb = pe.tile([128, 2 * QTW], f32, name="usb", tag="usb",
                              bufs=2)
                nc.vector.tensor_copy(usb[:], psu[:])
                psu = usb
                if LAMT_EPI:
                    # single-reciprocal O~ = U1 - (lam*d1/d2)*U2; the d1
                    # scale cancels in the rms.  Numerically worse: the two
                    # branches' reciprocal errors no longer cancel in the
                    # differential subtraction.
                    rec2 = pe.tile([64, QTW], f32, name="rec2", tag="rec2")
                    nc.vector.reciprocal(rec2[:], psu[64:128, QTW:2 * QTW])
                    lamt = pe.tile([64, QTW], f32, name="lamt", tag="lamt")
                    nc.vector.scalar_tensor_tensor(
                        lamt[:], rec2[:], lam, psu[64:128, 0:QTW],
                        ALU.mult, ALU.mult)
                    t2 = pe.tile([64, QTW], bf16, name="t2", tag="t2")
                    nc.vector.tensor_mul(t2[:], psu[0:64, QTW:2 * QTW],
                                         lamt[:])
                    nc.vector.tensor_sub(opair[pt][half:half + 64, :],
                                         psu[0:64, 0:QTW], t2[:])
                else:
                    # O = U1/d1 - lam*U2/d2: both reciprocals come from the
                    # same DVE approx, so their errors track (d1 ~= d2) and
                    # cancel in the differential subtraction
                    wri = pe.tile([64, 2 * QTW], f32, name="wri", tag="wri")
                    nc.vector.reciprocal(wri[:], psu[64:128, :])
                    t1 = pe.tile([64, QTW], bf16, name="t1", tag="t1")
                    nc.vector.tensor_mul(t1[:], psu[0:64, 0:QTW],
                                         wri[0:64, 0:QTW])
                    t2 = pe.tile([64, QTW], bf16, name="t2", tag="t2")
                    nc.vector.scalar_tensor_tensor(
                        t2[:], psu[0:64, QTW:2 * QTW], lam,
                        wri[0:64, QTW:2 * QTW], ALU.mult, ALU.mult)
                    nc.vector.tensor_sub(opair[pt][half:half + 64, :],
                                         t1[:], t2[:])
                # fused ssq contribution for the rms: row-sum of O~^2
                if j == 0:
                    ssqr = prm.tile([128, QTW], f32, name="ssqr", tag="ssqr")
                    pool_eng.memset(ssqr[:], 1.0)
                    state[(qt, "ssqr")] = ssqr
                ssqr = state[(qt, "ssqr")]
                osq = prm.tile([128, QTW], bf16, name="osq", tag="osq")
                nc.vector.tensor_mul(osq[half:half + 64, :],
                                     opair[pt][half:half + 64, :],
                                     opair[pt][half:half + 64, :])
                state["pend_ssq"] = (osq, ssqr, half, j)

            def emit_rms(qt, state, pts=(0, 1)):
                mark(f"rms{qt}")
                if state.get("pend_ssq") is not None:
                    osq_, ssqr_, half_, j_ = state.pop("pend_ssq")
                    psss = ps_.tile([1, QTW], f32, name="psss", tag="aux")
                    nc.tensor.matmul(psss[:],
                                     W['ones_sb'][half_:half_ + 64, 0:1],
                                     osq_[half_:half_ + 64, :],
                                     start=True, stop=True,
                                     skip_group_check=True)
                    nc.vector.tensor_copy(
                        ssqr_[32 * j_:32 * j_ + 1, 0:QTW], psss[0:1, :])
                opair = state[(qt, "op")]
                onq = state[(qt, "on")]
                ssqr = state[(qt, "ssqr")]
                # head j's ssq sits on partition 32j; one Ln + one Exp span
                # the whole partition range (ACT cost is free-size only; the
                # in-between rows hold memset filler)
                r0 = 0   # device activations start at partition 0
                r1 = 64 * pts[-1] + 33
                sqr = prm.tile([128, QTW], f32, name="sqr", tag="sqr")
                rmq = prm.tile([128, QTW], bf16, name="rmq", tag="rmq")
                if OLD_RMS:
                    for pt in pts:
                        for rr in (64 * pt, 64 * pt + 32):
                            nc.scalar.activation(sqr[rr:rr + 1, :],
                                                 ssqr[rr:rr + 1, :],
                                                 AF.Ln, scale=1.0 / 64.0,
                                                 bias=eps_sb[rr:rr + 1, 0:1])
                            nc.scalar.activation(rmq[rr:rr + 1, :],
                                                 sqr[rr:rr + 1, :],
                                                 AF.Exp, scale=-0.5)
                else:
                    nc.scalar.activation(sqr[r0:r1, :], ssqr[r0:r1, :],
                                         AF.Ln, scale=1.0 / 64.0,
                                         bias=eps_sb[r0:r1, 0:1])
                    nc.scalar.activation(rmq[r0:r1, :], sqr[r0:r1, :],
                                         AF.Exp, scale=-0.5)
                if DEBUG_DUMP and qt == 0:
                    dss = prm.tile([128, QTW], f32, name="dss", tag="dss")
                    nc.vector.tensor_copy(dss[:], ssqr[:])
                    nc.sync.dma_start(dbg["dbg_ssqr0"][:], dss[:])
                for pt in pts:
                    # partition_broadcast only honors a base-0 input on HW:
                    # stage the two factor rows onto partition 0 first
                    stg = prm.tile([1, 2 * QTW], bf16, name="stg", tag="stg",
                                   bufs=2)
                    nc.vector.tensor_copy(stg[0:1, 0:QTW],
                                          rmq[64 * pt:64 * pt + 1, :])
                    nc.vector.tensor_copy(stg[0:1, QTW:2 * QTW],
                                          rmq[64 * pt + 32:64 * pt + 33, :])
                    rsa = prm.tile([128, QTW], bf16, name="rsa", tag="rsa",
                                   bufs=2)
                    nc.gpsimd.partition_broadcast(rsa[:], stg[0:1, 0:QTW])
                    rsb = prm.tile([128, QTW], bf16, name="rsb", tag="rsb",
                                   bufs=2)
                    nc.gpsimd.partition_broadcast(rsb[:],
                                                  stg[0:1, QTW:2 * QTW])
                    # onq multiplies stay on DVE: they gate the wo matmuls
                    nc.vector.tensor_mul(onq[pt][0:64, :], opair[pt][0:64, :],
                                         rsa[0:64, :])
                    nc.vector.tensor_mul(onq[pt][64:128, :],
                                         opair[pt][64:128, :], rsb[64:128, :])
                if DEBUG_DUMP and qt == 0:
                    for nm, t_ in (("dbg_op0", opair[0]), ("dbg_on0", onq[0])):
                        d_ = prm.tile([128, 512], f32, name=f"e{nm}",
                                      tag="dbge", bufs=2)
                        nc.vector.tensor_copy(d_[:], t_[:])
                        nc.sync.dma_start(dbg[nm][:], d_[:])
                if DUMP_LITE and qt == 0:
                    for nm, t_ in (("dbg_ssqr0", ssqr), ("dbg_rmq0", rmq),
                                   ("dbg_on0", onq[0]), ("dbg_on1", onq[1])):
                        d_ = prm.tile([128, 512], f32, name=f"L{nm}",
                                      tag="dbgL", bufs=4)
                        nc.vector.tensor_copy(d_[:], t_[:])
                        nc.sync.dma_start(dbg[nm][:], d_[:])

            def emit_wo(qt, state, half):
                mark(f"wo{qt}.{half}")
                qlo, qhi = qt * QTW, (qt + 1) * QTW
                onq = state[(qt, "on")]
                oc0 = 4 * half
                ow = prm.tile([128, 4 * QTW], bf16, name="ow", tag="ow")
                for i, oc in enumerate(range(oc0, oc0 + 4)):
                    psw = ps_.tile([128, QTW], f32, name="psw", tag="aux")
                    nc.tensor.matmul(
                        psw[:], W['wo_sb'][0][:, oc * 128:(oc + 1) * 128],
                        onq[0][:], start=True, stop=False)
                    nc.tensor.matmul(
                        psw[:], W['wo_sb'][1][:, oc * 128:(oc + 1) * 128],
                        onq[1][:], start=False, stop=True)
                    nc.vector.tensor_copy(ow[:, i * QTW:(i + 1) * QTW],
                                          psw[:])
                nc.sync.dma_start(
                    out_pt[oc0 * 128:(oc0 + 4) * 128,
                           qlo:qhi].rearrange("(b p) c -> p b c", b=4),
                    ow[:].rearrange("p (b c) -> p b c", b=4))

            def emit_wo_tail(qt, state):
                # last q tile: 8 parallel psum accumulators (attention's psS
                # and psU banks are free by now) so all 8 onq[0] matmuls run
                # while the rms of the second head pair is still in flight
                mark(f"wo{qt}.T")
                qlo, qhi = qt * QTW, (qt + 1) * QTW
                onq = state[(qt, "on")]
                psws = []
                bigA = ps_.tile([128, 2 * QTW], f32, name="woTa", tag="psS")
                bigB = ps_.tile([128, 2 * QTW], f32, name="woTb", tag="psS")
                bigU = ps_.tile([128, 2 * QTW], f32, name="woTu", tag="psU",
                                bufs=1)
                for big in (bigA, bigB, bigU):
                    psws.append(big[:, 0:QTW])
                    psws.append(big[:, QTW:2 * QTW])
                for _ in range(2):
                    aux = ps_.tile([128, QTW], f32, name="woTx", tag="aux")
                    psws.append(aux[:])
                for oc in range(8):
                    nc.tensor.matmul(
                        psws[oc], W['wo_sb'][0][:, oc * 128:(oc + 1) * 128],
                        onq[0][:], start=True, stop=False,
                        skip_group_check=True)
                ow = prm.tile([128, 8 * QTW], bf16, name="owT", tag="owT")
                for oc in range(8):
                    nc.tensor.matmul(
                        psws[oc], W['wo_sb'][1][:, oc * 128:(oc + 1) * 128],
                        onq[1][:], start=False, stop=True,
                        skip_group_check=True)
                    nc.vector.tensor_copy(ow[:, oc * QTW:(oc + 1) * QTW],
                                          psws[oc])
                    if oc == 3 or oc == 7:
                        half = oc // 4
                        nc.sync.dma_start(
                            out_pt[half * 512:(half + 1) * 512,
                                   qlo:qhi].rearrange("(b p) c -> p b c",
                                                      b=4),
                            ow[:, half * 2048:(half + 1) * 2048].rearrange(
                                "p (b c) -> p b c", b=4))

            state = {}
            emit_weight_loads0(state)
            for _ in proj_units(0, state):
                pass
            for qt in range(NQT):
                gen = None
                if qt < NQT - 1:
                    emit_xt_loads(qt + 1, state)
                    gen = proj_units(qt + 1, state)
                for j in range(NHL):
                    emit_att_head(qt, j, state, filler=gen)
                    if j == 1 and qt > 0:
                        emit_rms(qt - 1, state)
                    if j == 1 and qt == NQT - 1:
                        emit_rms(qt, state, pts=(0,))
                    if j == 2 and qt > 0:
                        emit_wo(qt - 1, state, half=0)
                    if j == 3 and qt > 0:
                        emit_wo(qt - 1, state, half=1)
                if gen is not None:
                    for _ in gen:
                        pass
            emit_rms(NQT - 1, state, pts=(1,))
            emit_wo_tail(NQT - 1, state)
    nc.compile()
    return nc


def get_program(lam: float):
    key = (round(float(lam), 9), FP8_QKV, SIMPLE_IN, POOL_OFF, DEBUG_DUMP,
           LAMT_EPI, OLD_RMS, OLD_DIAG, OLD_ROPE, DUMP_LITE)
    if key not in _prog_cache:
        _prog_cache[key] = _build_program(float(lam))
    return _prog_cache[key]


def _bf16():
    import ml_dtypes
    return ml_dtypes.bfloat16


def _fp8():
    import ml_dtypes
    return ml_dtypes.float8_e4m3


def _perm_mat():
    # For OLD_ROPE: psr = perm.T @ q with psr[d] = rot(q)[d]/sgn2(d) where
    # sgn2 is the sign folded into sin_t; both cases come out -1.
    p = np.zeros((128, 128), np.float32)
    for o in range(128):
        if o % 64 < 32:
            p[o + 32, o] = -1.0
        else:
            p[o - 32, o] = -1.0
    return p


def _host_inputs(x, rope_cos, rope_sin, Wq, Wk, Wv, Wo, subln_w, lam):
    bf = _bf16()
    in_t = _fp8() if FP8_QKV else bf
    cos_t = np.ascontiguousarray(np.tile(rope_cos.T, (4, 1))).astype(bf)
    # row-permuted signed sin: row p holds sgn(partner)*sin[p%32] where
    # partner = p+-32 within each 64-block, so the shifted rot-multiply
    # reads q_sb and sin at the SAME base partition.  qs[d] = -q[d+32]*sin[d]
    # for d%64<32 (else +q[d-32]*sin[d]) => sin_t[p] = -sin[p%32] for
    # p%64>=32, +sin[p%32] otherwise.
    sgn = np.where((np.arange(128) % 64) < 32, 1.0, -1.0)[:, None]
    sin_t = np.ascontiguousarray(
        np.tile(rope_sin.T, (4, 1)) * sgn).astype(bf)
    tri = np.triu(np.ones((128, 128), np.float32)).astype(bf)
    sub4 = np.tile(subln_w.astype(np.float32), 4)[:, None]

    in_maps = []
    for c in range(8):
        b, g = c // 4, c % 4
        xtc = np.ascontiguousarray(x[b].T).astype(in_t)
        cols = []
        for j in range(NHL):
            h = 4 * g + j
            cols.append(Wq[:, h * 64:(h + 1) * 64])
            cols.append(Wq[:, (H + h) * 64:(H + h + 1) * 64])
        wq_c = np.ascontiguousarray(np.concatenate(cols, axis=1)).astype(in_t)
        wk_c = np.ascontiguousarray(np.concatenate(
            [Wk[:, g * 64:(g + 1) * 64], Wk[:, (KV + g) * 64:(KV + g + 1) * 64]],
            axis=1)).astype(in_t)
        wv_c = np.ascontiguousarray(Wv[:, g * 64:(g + 1) * 64]).astype(in_t)
        wo_c = np.ascontiguousarray(
            Wo[g * 256:(g + 1) * 256, :] * sub4).astype(bf)
        in_maps.append({
            "xt": xtc, "wq": wq_c, "wk": wk_c, "wv": wv_c, "wo": wo_c,
            "perm": _perm_mat().astype(bf),
            "cos_t": cos_t, "sin_t": sin_t, "trimask": tri,
        })
    return in_maps


def _compute_lam(lambda_q1, lambda_k1, lambda_q2, lambda_k2):
    li = 0.8 - 0.6 * math.exp(-0.3)
    l1 = np.exp(np.dot(lambda_q1.astype(np.float32), lambda_k1.astype(np.float32)))
    l2 = np.exp(np.dot(lambda_q2.astype(np.float32), lambda_k2.astype(np.float32)))
    return float(l1 - l2 + li)


def _numpy_reference(x, rope_cos, rope_sin, attention_mask, Wq, Wk, Wv, Wo,
                     lambda_q1, lambda_k1, lambda_q2, lambda_k2, subln_w):
    """Pure-numpy fallback, only used if the mask is not the expected causal one."""
    bsz, seq_len, _ = x.shape

    def rope(t):
        c = np.concatenate([rope_cos, rope_cos], axis=-1)[None, None]
        s = np.concatenate([rope_sin, rope_sin], axis=-1)[None, None]
        t1, t2 = np.split(t, 2, axis=-1)
        rot = np.concatenate([-t2, t1], axis=-1)
        return t * c + rot * s

    q = (x @ Wq).reshape(bsz, seq_len, 2 * H, D)
    q1 = np.transpose(q[:, :, :H], (0, 2, 1, 3))
    q2 = np.transpose(q[:, :, H:], (0, 2, 1, 3))
    k = (x @ Wk).reshape(bsz, seq_len, 2 * KV, D)
    k1 = np.transpose(k[:, :, :KV], (0, 2, 1, 3))
    k2 = np.transpose(k[:, :, KV:], (0, 2, 1, 3))
    v = np.transpose((x @ Wv).reshape(bsz, seq_len, KV, D), (0, 2, 1, 3))
    q1, q2, k1, k2 = rope(q1), rope(q2), rope(k1), rope(k2)
    gr = H // KV
    k1 = np.repeat(k1, gr, axis=1)
    k2 = np.repeat(k2, gr, axis=1)
    v = np.repeat(v, gr, axis=1)
    scale = 1.0 / math.sqrt(D)

    def smax(a):
        a = a - a.max(axis=-1, keepdims=True)
        e = np.exp(a)
        return e / e.sum(axis=-1, keepdims=True)

    a1 = smax(np.einsum("bhqd,bhkd->bhqk", q1, k1) * scale + attention_mask)
    a2 = smax(np.einsum("bhqd,bhkd->bhqk", q2, k2) * scale + attention_mask)
    lam = _compute_lam(lambda_q1, lambda_k1, lambda_q2, lambda_k2)
    attn = a1 - lam * a2
    out = np.einsum("bhqk,bhkd->bhqd", attn, v)
    inv = 1.0 / np.sqrt(np.mean(out * out, axis=-1, keepdims=True) + EPS)
    out = out * inv * subln_w
    out = np.transpose(out, (0, 2, 1, 3)).reshape(bsz, seq_len, HS)
    return (out @ Wo).astype(np.float32)


LAST_RESULT = None


def kernel(x, rope_cos, rope_sin, attention_mask, Wq, Wk, Wv, Wo,
           lambda_q1, lambda_k1, lambda_q2, lambda_k2, subln_w):
    global LAST_RESULT
    x = np.asarray(x, np.float32)
    kk, qq = np.arange(S)[:, None], np.arange(S)[None, :]
    causal = np.where(qq <= kk, 0.0, NEG).astype(np.float32)[None, None]
    am = np.asarray(attention_mask, np.float32)
    if am.shape != (1, 1, S, S) or not np.array_equal(am, causal):
        return _numpy_reference(x, rope_cos, rope_sin, am, Wq, Wk, Wv, Wo,
                                lambda_q1, lambda_k1, lambda_q2, lambda_k2,
                                subln_w)

    lam = _compute_lam(lambda_q1, lambda_k1, lambda_q2, lambda_k2)
    nc = get_program(lam)
    in_maps = _host_inputs(x, np.asarray(rope_cos, np.float32),
                           np.asarray(rope_sin, np.float32),
                           np.asarray(Wq, np.float32), np.asarray(Wk, np.float32),
                           np.asarray(Wv, np.float32), np.asarray(Wo, np.float32),
                           np.asarray(subln_w, np.float32), lam)
    res = bass_utils.run_bass_kernel_spmd(nc, in_maps, core_ids=list(range(8)))
    LAST_RESULT = res
    y = np.zeros((B, S, HS), np.float32)
    for c in range(8):
        y[c // 4] += res.results[c]["out_pt"].T.astype(np.float32)
    return y
